# revision 35
# baseline (speedup 1.0000x reference)
"""Trainium2 Bass kernel for GQA attention layer (B=1, S=2048, H=4096,
32 Q heads / 8 KV heads, head_dim 128, RoPE with arbitrary tables).

Sharding: tensor-parallel over heads across 8 NeuronCores — core c gets
Q heads 4c..4c+3 and KV head c (Wq/Wk/Wv column shards, Wo row shard).
Each core computes its partial o_proj output [2048, 4096]; the host sums
the 8 partials (equivalent of the all-reduce).

Per-core compute (all matmuls bf16 with fp32 PSUM accumulation):
  Phase A: qT/kT/vT = W.T @ hs.T in [d, s] layout (N=512 matmuls),
           RoPE applied via rotate-half partition swap (SBUF-to-SBUF DMA)
           + elementwise DVE ops; v transposed to [s, d] chunks on PE.
  Phase B: flash-style causal attention per (head, q-range, k-tile):
           scoresT[k,q] = kT.T @ qT, probsT = exp(scale*scores),
           attn_oT[d,q] += v[k,d].T @ probsT, denom[1,q] += ones.T @
           probsT; diagonal k-tiles narrowed to the unmasked column range
           with a single triangular 128-col mask multiply. Normalization
           via fast reciprocal + gpsimd partition_broadcast + DVE mul.
           No max-subtraction (scores are bounded; fp32 exp is exact
           enough).
  Phase C: partial o_proj [s, hidden] = attn_oT.T @ Wo_shard.
"""

import sys
from contextlib import ExitStack

sys.path.insert(0, "/opt/trn_rl_repo")

import numpy as np
import ml_dtypes

import concourse.bass as bass
import concourse.bacc as bacc
import concourse.mybir as mybir
import concourse.tile as tile
from concourse.bass_utils import run_bass_kernel_spmd
from concourse import bass_isa
from concourse.masks import make_identity

BF16 = mybir.dt.bfloat16
F32 = mybir.dt.float32

N_CORES = 8
S = 2048
HID = 4096
D = 128
NQ = 4  # q heads per core
KC = HID // 128  # 32 hidden-dim chunks
NQR = S // 512  # 4 q ranges of 512
NST = S // 128  # 16 s-tiles of 128
NHO = HID // 512  # 8 output column tiles of 512
SCALE = 1.0 / float(np.sqrt(D))

_CACHE: dict = {}


def _build_nc():
    nc = bacc.Bacc(None, target_bir_lowering=False, debug=False)

    hst_d = nc.dram_tensor("hst", [NQR, 128, KC, 512], BF16, kind="ExternalInput")
    wq_d = nc.dram_tensor("wq", [NQ, 128, KC, D], BF16, kind="ExternalInput")
    wk_d = nc.dram_tensor("wk", [128, KC, D], BF16, kind="ExternalInput")
    wv_d = nc.dram_tensor("wv", [128, KC, D], BF16, kind="ExternalInput")
    wo_d = nc.dram_tensor("wo", [128, NQ, HID], BF16, kind="ExternalInput")
    cos_d = nc.dram_tensor("cos2", [128, S], F32, kind="ExternalInput")
    sin_d = nc.dram_tensor("sin2", [128, S], F32, kind="ExternalInput")
    out_d = nc.dram_tensor("out", [S, HID], F32, kind="ExternalOutput")

    with tile.TileContext(nc) as tc, ExitStack() as stack:
        # ---- pools that live the whole kernel ----
        const = stack.enter_context(tc.tile_pool(name="const", bufs=1))
        act = stack.enter_context(tc.tile_pool(name="act", bufs=1))
        qt_sb = [
            act.tile([128, S], BF16, tag=f"qt{h}", name=f"qt{h}") for h in range(NQ)
        ]
        kt_sb = act.tile([128, S], BF16, tag="kt")
        vt_sb = act.tile([128, S], BF16, tag="vt")
        v_sb = act.tile([128, NST, 128], BF16, tag="v")  # [s,d] chunks per k-tile
        attn_sb = [
            act.tile([128, S], BF16, tag=f"attn{h}", name=f"attn{h}")
            for h in range(NQ)
        ]
        # B-phase SBUF pools are allocated up-front (NOT from space reused
        # from the A-phase pools) — otherwise the first B eviction picks up
        # a WAR dependency on the tail of phase A and the PE/DVE/GPSIMD
        # engines form a multi-microsecond circular stall.
        probs_p = stack.enter_context(tc.tile_pool(name="probs", bufs=3))
        den_p = stack.enter_context(tc.tile_pool(name="den", bufs=2))
        bcast_p = stack.enter_context(tc.tile_pool(name="bcast", bufs=2))

        # ================= Phase A: QKV projections + RoPE =================
        with (
            tc.tile_pool(name="wqkv", bufs=1) as wqkv,
            tc.tile_pool(name="hstp", bufs=2) as hstp,
            tc.tile_pool(name="rope", bufs=2) as rope,
            tc.tile_pool(name="psA", bufs=3, space="PSUM") as psA,
            tc.tile_pool(name="psT", bufs=2, space="PSUM") as psT,
        ):
            # DMA order matters at startup: get hst[0] + wk + rope tables in
            # first so the k-projection (first job) can start ASAP.
            hst_tiles = []
            hst_t0 = hstp.tile([128, KC, 512], BF16, tag="hst", name="hst0")
            for r in range(4):
                nc.sync.dma_start(
                    hst_t0[:, r * 8 : (r + 1) * 8, :],
                    hst_d[0, :, r * 8 : (r + 1) * 8, :],
                )
            hst_tiles.append(hst_t0)
            wk_sb = wqkv.tile([128, KC, D], BF16)
            nc.sync.dma_start(wk_sb[:], wk_d[:])
            cos_sb = const.tile([128, S], F32)
            sin_sb = const.tile([128, S], F32)
            nc.sync.dma_start(cos_sb[:], cos_d[:])
            nc.sync.dma_start(sin_sb[:], sin_d[:])
            wv_sb = wqkv.tile([128, KC, D], BF16)
            nc.sync.dma_start(wv_sb[:], wv_d[:])
            wq_sb = [
                wqkv.tile([128, KC, D], BF16, tag=f"wq{h}", name=f"wq{h}")
                for h in range(NQ)
            ]
            for h in range(NQ):
                nc.sync.dma_start(wq_sb[h][:], wq_d[h])

            identity = const.tile([128, 128], BF16)
            make_identity(nc, identity[:])
            ones = const.tile([128, 1], BF16)
            nc.gpsimd.memset(ones[:], 1.0)
            ones_f = const.tile([128, 1], F32)
            nc.gpsimd.memset(ones_f[:], 1.0)
            # triangular mask for the diagonal 128x128 subtile: rows are k,
            # cols are q; keep q >= k.
            tri = const.tile([128, 128], BF16)
            nc.gpsimd.memset(tri[:], 1.0)
            nc.gpsimd.affine_select(
                out=tri[:],
                in_=tri[:],
                pattern=[[1, 128]],
                compare_op=mybir.AluOpType.is_ge,
                fill=0.0,
                base=0,
                channel_multiplier=-1,
            )

            def rope_evict(ps, dst_tile, qr):
                """dst[0:64]  = x0*cos - x1*sin
                dst[64:128] = x1*cos + x0*sin   (x0=ps[0:64], x1=ps[64:128])"""
                sl = slice(qr * 512, (qr + 1) * 512)
                raw = rope.tile([128, 512], F32, tag="raw")
                nc.vector.tensor_copy(raw[:], ps[:])
                swp = rope.tile([128, 512], F32, tag="swp")
                nc.sync.dma_start(swp[0:64, :], raw[64:128, :])
                nc.sync.dma_start(swp[64:128, :], raw[0:64, :])
                # in-place: raw *= cos, swp *= sin
                nc.vector.tensor_mul(raw[:], raw[:], cos_sb[:, sl])
                nc.vector.tensor_mul(swp[:], swp[:], sin_sb[:, sl])
                nc.vector.tensor_sub(dst_tile[0:64, sl], raw[0:64, :], swp[0:64, :])
                nc.vector.tensor_add(
                    dst_tile[64:128, sl], raw[64:128, :], swp[64:128, :]
                )

            for qr in range(NQR):
                if qr + 1 < NQR:
                    nxt = hstp.tile([128, KC, 512], BF16, tag="hst", name=f"hst{qr+1}")
                    nc.sync.dma_start(nxt[:], hst_d[qr + 1])
                    hst_tiles.append(nxt)
                hst_t = hst_tiles[qr]
                # k and v first (their weights arrive first)
                jobs = [("k", 0), ("v", 0)] + [("q", h) for h in range(NQ)]
                for kind, h in jobs:
                    ps = psA.tile([128, 512], F32)
                    for c in range(KC):
                        if kind == "q":
                            lhsT = wq_sb[h][:, c, :]
                        elif kind == "k":
                            lhsT = wk_sb[:, c, :]
                        else:
                            lhsT = wv_sb[:, c, :]
                        nc.tensor.matmul(
                            ps[:],
                            lhsT,
                            hst_t[:, c, :],
                            start=(c == 0),
                            stop=(c == KC - 1),
                        )
                    if kind == "q":
                        rope_evict(ps, qt_sb[h], qr)
                    elif kind == "k":
                        rope_evict(ps, kt_sb, qr)
                    else:
                        sl = slice(qr * 512, (qr + 1) * 512)
                        nc.vector.tensor_copy(vt_sb[:, sl], ps[:])
                # transpose this qr's v slice into [s, d] chunks
                for kt in range(qr * 4, qr * 4 + 4):
                    pst = psT.tile([128, 128], BF16)
                    nc.tensor.transpose(
                        pst[:], vt_sb[:, kt * 128 : (kt + 1) * 128], identity[:]
                    )
                    nc.vector.tensor_copy(v_sb[:, kt, :], pst[:])

        # ========== Phase B: causal attention (+ interleaved o_proj) ==========
        wo_pool = stack.enter_context(tc.tile_pool(name="wo", bufs=1))
        wo_sb = wo_pool.tile([128, NQ, HID], BF16)
        nc.sync.dma_start(wo_sb[:], wo_d[:])
        ostage = stack.enter_context(tc.tile_pool(name="ostage", bufs=6))

        # k-tiles are processed in PAIRS: one [128, 1024] PSUM scores tile
        # holds two k-tiles' scoresT so one ACT exp instruction covers both
        # (the 352-cycle per-ACTIVATE overhead would otherwise make ACT the
        # phase-B pacer). Diagonal pairs use two column-narrowed exps.
        #
        # Phase B is exp-throughput-paced on ACT, leaving PE bubbles; o_proj
        # (phase C) matmuls for already-completed q-ranges are interleaved
        # into those bubbles as PE filler. PSUM: scores pairs 2x2 banks +
        # o_proj accumulators 2 + attention output 2 = 8 banks.
        with (
            tc.tile_pool(name="psS", bufs=2, space="PSUM") as psS,
            tc.tile_pool(name="psO", bufs=2, space="PSUM") as psO,
        ):
            # ---- o_proj work-unit generator ----
            def c_units():
                for qrC in range(NQR):
                    for st in range(qrC * 4, qrC * 4 + 4):
                        for ho in range(NHO):
                            yield ("alloc", qrC, st, ho)
                            for h in range(NQ):
                                yield ("mm", qrC, st, ho, h)
                            yield ("evict", qrC, st, ho)

            c_state = {"gen": c_units(), "pending": None, "tile": None,
                       "done": set(), "open": None}

            def emit_c(n_mms, qr_done, evict_engine="v"):
                emitted = 0
                while emitted < n_mms:
                    unit = c_state["pending"] or next(c_state["gen"], None)
                    c_state["pending"] = None
                    if unit is None:
                        return False
                    if unit[1] > qr_done:
                        c_state["pending"] = unit
                        return False
                    if unit[0] == "alloc":
                        _, _, st, ho = unit
                        c_state["open"] = (st, ho)
                        c_state["tile"] = psS.tile(
                            [128, 512], F32, tag="c", bufs=2, name=f"c{st}_{ho}"
                        )
                    elif unit[0] == "mm":
                        _, _, st, ho, h = unit
                        nc.tensor.matmul(
                            c_state["tile"][:],
                            attn_sb[h][:, st * 128 : (st + 1) * 128],
                            wo_sb[:, h, ho * 512 : (ho + 1) * 512],
                            start=(h == 0),
                            stop=(h == NQ - 1),
                            skip_group_check=True,
                        )
                        emitted += 1
                    else:
                        _, _, st, ho = unit
                        c_state["done"].add((st, ho))
                        stg = ostage.tile([128, 512], F32, tag="stg")
                        nc.vector.tensor_copy(stg[:], c_state["tile"][:])
                        nc.sync.dma_start(
                            out_d[
                                st * 128 : (st + 1) * 128,
                                ho * 512 : (ho + 1) * 512,
                            ],
                            stg[:],
                        )
                return True

            for qr in range(NQR):
                n_kt = 4 * (qr + 1)
                n_pair = n_kt // 2
                qsl = slice(qr * 512, (qr + 1) * 512)

                def c0_of(kt, qr=qr):
                    p_idx = kt - 4 * qr
                    return 128 * p_idx if p_idx > 0 else 0

                for h in range(NQ):
                    ps_o = psO.tile([128, 512], F32, tag="o", name=f"o{qr}_{h}")
                    den_acc = den_p.tile(
                        [128, 512], F32, tag="da", name=f"da{qr}_{h}"
                    )
                    pair_tiles = {}

                    def mm_scores_pair(j, qr=qr, h=h, pair_tiles=pair_tiles):
                        ps_s = psS.tile(
                            [128, 1024], F32, tag="s", name=f"s{qr}_{h}_{j}"
                        )
                        for idx in range(2):
                            kt = 2 * j + idx
                            c0 = c0_of(kt)
                            nc.tensor.matmul(
                                ps_s[:, idx * 512 + c0 : (idx + 1) * 512],
                                kt_sb[:, kt * 128 : (kt + 1) * 128],
                                qt_sb[h][:, qr * 512 + c0 : (qr + 1) * 512],
                                start=True,
                                stop=True,
                            )
                        pair_tiles[j] = ps_s

                    mm_scores_pair(0)
                    for j in range(n_pair):
                        if j + 1 < n_pair:
                            mm_scores_pair(j + 1)
                        emit_c(2, qr - 1)
                        ps_s = pair_tiles.pop(j)
                        pt = probs_p.tile(
                            [128, 1024], BF16, tag="pt", name=f"pt{qr}_{h}_{j}"
                        )
                        if 2 * j + 1 < 4 * qr:
                            # fully off-diagonal pair: one batched exp
                            nc.scalar.activation(
                                pt[:],
                                ps_s[:],
                                mybir.ActivationFunctionType.Exp,
                                scale=SCALE,
                            )
                        else:
                            for idx in range(2):
                                kt = 2 * j + idx
                                c0 = c0_of(kt)
                                nc.scalar.activation(
                                    pt[:, idx * 512 + c0 : (idx + 1) * 512],
                                    ps_s[:, idx * 512 + c0 : (idx + 1) * 512],
                                    mybir.ActivationFunctionType.Exp,
                                    scale=SCALE,
                                )
                                # triangular mask on the diagonal subtile
                                nc.vector.tensor_mul(
                                    pt[:, idx * 512 + c0 : idx * 512 + c0 + 128],
                                    pt[:, idx * 512 + c0 : idx * 512 + c0 + 128],
                                    tri[:],
                                )
                        for idx in range(2):
                            kt = 2 * j + idx
                            c0 = c0_of(kt)
                            nc.tensor.matmul(
                                ps_o[:, c0:512],
                                v_sb[:, kt, :],
                                pt[:, idx * 512 + c0 : (idx + 1) * 512],
                                start=(kt == 0),
                                stop=(kt == n_kt - 1),
                                skip_group_check=True,
                            )
                        # denominator: fold the pair halves (bf16) and
                        # accumulate on DVE in f32 — no per-pair PE matmul
                        pts = probs_p.tile(
                            [128, 512], BF16, tag="pts", name=f"pts{qr}_{h}_{j}"
                        )
                        c0a, c0b = c0_of(2 * j), c0_of(2 * j + 1)
                        if c0b > c0a:
                            nc.vector.tensor_copy(
                                pts[:, c0a:c0b], pt[:, c0a:c0b]
                            )
                        nc.vector.tensor_add(
                            pts[:, c0b:512],
                            pt[:, c0b:512],
                            pt[:, 512 + c0b : 1024],
                        )
                        if j == 0:
                            nc.vector.tensor_copy(den_acc[:], pts[:])
                        else:
                            nc.vector.tensor_add(
                                den_acc[:, c0a:512],
                                den_acc[:, c0a:512],
                                pts[:, c0a:512],
                            )
                    emit_c(2, qr - 1)
                    # partition-reduce den_acc with one fp32 matmul into the
                    # (dead) last scores psum tile, then recip + broadcast
                    nc.tensor.matmul(
                        ps_s[0:1, 0:512],
                        ones_f[:],
                        den_acc[:],
                        start=True,
                        stop=True,
                        skip_group_check=True,
                    )
                    recip = den_p.tile(
                        [1, 512], F32, tag="recip", name=f"rc{qr}_{h}"
                    )
                    nc.vector.reciprocal_approx_fast(
                        out=recip[:], in_=ps_s[0:1, 0:512]
                    )
                    bc = bcast_p.tile([128, 512], F32, tag="bc")
                    nc.gpsimd.partition_broadcast(bc[:], recip[:])
                    nc.vector.tensor_mul(attn_sb[h][:, qsl], ps_o[:], bc[:])

            # finish any partially-emitted o_proj group before leaving
            # this PSUM layout
            if c_state["open"] is not None and c_state["open"] not in c_state["done"]:
                emit_c(NQ, NQR - 1)
            rem = [
                (st, ho)
                for st in range(NST)
                for ho in range(NHO)
                if (st, ho) not in c_state["done"]
            ]

        # ---- drain the remaining o_proj work, classic double-buffered ----
        with tc.tile_pool(name="psC", bufs=8, space="PSUM") as psC:
            by_st = {}
            for st, ho in rem:
                by_st.setdefault(st, []).append(ho)
            alt = 0
            for st, hos in by_st.items():
                ssl = slice(st * 128, (st + 1) * 128)
                for g in range(0, len(hos), 4):
                    chunk = hos[g : g + 4]
                    ps_c = [
                        psC.tile([128, 512], F32, tag="c2", name=f"d{st}_{ho}")
                        for ho in chunk
                    ]
                    for h in range(NQ):
                        lhsT = attn_sb[h][:, ssl]
                        for i, ho in enumerate(chunk):
                            nc.tensor.matmul(
                                ps_c[i][:],
                                lhsT,
                                wo_sb[:, h, ho * 512 : (ho + 1) * 512],
                                start=(h == 0),
                                stop=(h == NQ - 1),
                                skip_group_check=True,
                            )
                    for i, ho in enumerate(chunk):
                        stg = ostage.tile([128, 512], F32, tag="stg")
                        if alt % 2 == 0:
                            nc.vector.tensor_copy(stg[:], ps_c[i][:])
                        else:
                            nc.scalar.copy(stg[:], ps_c[i][:])
                        alt += 1
                        nc.sync.dma_start(
                            out_d[ssl, ho * 512 : (ho + 1) * 512], stg[:]
                        )

    nc.compile()
    return nc


def _get_nc():
    if "nc" not in _CACHE:
        _CACHE["nc"] = _build_nc()
    return _CACHE["nc"]


def _bf16(x):
    return np.ascontiguousarray(x.astype(ml_dtypes.bfloat16))


def _prep_in_maps(hidden_states, sin_table, cos_table, Wq, Wk, Wv, Wo):
    hs0 = np.asarray(hidden_states, np.float32).reshape(S, HID)
    # hst[qr, p, c, s] = hs0[qr*512 + s, c*128 + p]
    hst = _bf16(hs0.reshape(NQR, 512, KC, 128).transpose(0, 3, 2, 1))
    cosT = np.asarray(cos_table, np.float32).T  # [64, S]
    sinT = np.asarray(sin_table, np.float32).T
    cos2 = np.ascontiguousarray(np.concatenate([cosT, cosT], 0))  # [128, S]
    sin2 = np.ascontiguousarray(np.concatenate([sinT, sinT], 0))
    Wq = np.asarray(Wq, np.float32)
    Wk = np.asarray(Wk, np.float32)
    Wv = np.asarray(Wv, np.float32)
    Wo = np.asarray(Wo, np.float32)

    in_maps = []
    for c in range(N_CORES):
        wq_c = Wq[:, c * 512 : (c + 1) * 512]  # 4 q heads
        wk_c = Wk[:, c * 128 : (c + 1) * 128]  # 1 kv head
        wv_c = Wv[:, c * 128 : (c + 1) * 128]
        wo_c = Wo[c * 512 : (c + 1) * 512, :]  # matching rows
        # wq per-head-major: [h, p, c, d] with element Wq_c[c*128+p, h*128+d]
        wq_l = wq_c.reshape(KC, 128, NQ, D).transpose(2, 1, 0, 3)
        in_maps.append(
            {
                "hst": hst,
                "wq": _bf16(wq_l),
                "wk": _bf16(wk_c.reshape(KC, 128, D).swapaxes(0, 1)),
                "wv": _bf16(wv_c.reshape(KC, 128, D).swapaxes(0, 1)),
                "wo": _bf16(wo_c.reshape(NQ, 128, HID).swapaxes(0, 1)),
                "cos2": cos2,
                "sin2": sin2,
            }
        )
    return in_maps


def run(trace=False, **inputs):
    nc = _get_nc()
    in_maps = _prep_in_maps(**inputs)
    res = run_bass_kernel_spmd(
        nc, in_maps, core_ids=list(range(N_CORES)), trace=trace
    )
    partials = np.stack([res.results[c]["out"] for c in range(N_CORES)])
    out = partials.sum(axis=0, dtype=np.float32).reshape(1, S, HID)
    return out, res


def kernel(**inputs):
    out, _ = run(trace=False, **inputs)
    return out


# revision 36
# speedup vs baseline: 1.0070x; 1.0070x over previous
"""Trainium2 Bass kernel for GQA attention layer (B=1, S=2048, H=4096,
32 Q heads / 8 KV heads, head_dim 128, RoPE with arbitrary tables).

Sharding: tensor-parallel over heads across 8 NeuronCores — core c gets
Q heads 4c..4c+3 and KV head c (Wq/Wk/Wv column shards, Wo row shard).
Each core computes its partial o_proj output [2048, 4096]; the host sums
the 8 partials (equivalent of the all-reduce).

Per-core compute (all matmuls bf16 with fp32 PSUM accumulation):
  Phase A: qT/kT/vT = W.T @ hs.T in [d, s] layout (N=512 matmuls),
           RoPE applied via rotate-half partition swap (SBUF-to-SBUF DMA)
           + elementwise DVE ops; v transposed to [s, d] chunks on PE.
  Phase B: flash-style causal attention per (head, q-range, k-tile):
           scoresT[k,q] = kT.T @ qT, probsT = exp(scale*scores),
           attn_oT[d,q] += v[k,d].T @ probsT, denom[1,q] += ones.T @
           probsT; diagonal k-tiles narrowed to the unmasked column range
           with a single triangular 128-col mask multiply. Normalization
           via fast reciprocal + gpsimd partition_broadcast + DVE mul.
           No max-subtraction (scores are bounded; fp32 exp is exact
           enough).
  Phase C: partial o_proj [s, hidden] = attn_oT.T @ Wo_shard.
"""

import sys
from contextlib import ExitStack

sys.path.insert(0, "/opt/trn_rl_repo")

import numpy as np
import ml_dtypes

import concourse.bass as bass
import concourse.bacc as bacc
import concourse.mybir as mybir
import concourse.tile as tile
from concourse.bass_utils import run_bass_kernel_spmd
from concourse import bass_isa
from concourse.masks import make_identity

BF16 = mybir.dt.bfloat16
F32 = mybir.dt.float32

N_CORES = 8
S = 2048
HID = 4096
D = 128
NQ = 4  # q heads per core
KC = HID // 128  # 32 hidden-dim chunks
NQR = S // 512  # 4 q ranges of 512
NST = S // 128  # 16 s-tiles of 128
NHO = HID // 512  # 8 output column tiles of 512
SCALE = 1.0 / float(np.sqrt(D))

_CACHE: dict = {}


def _build_nc():
    nc = bacc.Bacc(None, target_bir_lowering=False, debug=False)

    hst_d = nc.dram_tensor("hst", [NQR, 128, KC, 512], BF16, kind="ExternalInput")
    wq_d = nc.dram_tensor("wq", [NQ, 128, KC, D], BF16, kind="ExternalInput")
    wk_d = nc.dram_tensor("wk", [128, KC, D], BF16, kind="ExternalInput")
    wv_d = nc.dram_tensor("wv", [128, KC, D], BF16, kind="ExternalInput")
    wo_d = nc.dram_tensor("wo", [128, NQ, HID], BF16, kind="ExternalInput")
    cos_d = nc.dram_tensor("cos2", [128, S], F32, kind="ExternalInput")
    sin_d = nc.dram_tensor("sin2", [128, S], F32, kind="ExternalInput")
    out_d = nc.dram_tensor("out", [S, HID], F32, kind="ExternalOutput")

    with tile.TileContext(nc) as tc, ExitStack() as stack:
        # ---- pools that live the whole kernel ----
        const = stack.enter_context(tc.tile_pool(name="const", bufs=1))
        act = stack.enter_context(tc.tile_pool(name="act", bufs=1))
        qt_sb = [
            act.tile([128, S], BF16, tag=f"qt{h}", name=f"qt{h}") for h in range(NQ)
        ]
        kt_sb = act.tile([128, S], BF16, tag="kt")
        vt_sb = act.tile([128, S], BF16, tag="vt")
        v_sb = act.tile([128, NST, 128], BF16, tag="v")  # [s,d] chunks per k-tile
        attn_sb = [
            act.tile([128, S], BF16, tag=f"attn{h}", name=f"attn{h}")
            for h in range(NQ)
        ]
        # B-phase SBUF pools are allocated up-front (NOT from space reused
        # from the A-phase pools) — otherwise the first B eviction picks up
        # a WAR dependency on the tail of phase A and the PE/DVE/GPSIMD
        # engines form a multi-microsecond circular stall.
        probs_p = stack.enter_context(tc.tile_pool(name="probs", bufs=3))
        den_p = stack.enter_context(tc.tile_pool(name="den", bufs=2))
        bcast_p = stack.enter_context(tc.tile_pool(name="bcast", bufs=2))

        # ================= Phase A: QKV projections + RoPE =================
        with (
            tc.tile_pool(name="wqkv", bufs=1) as wqkv,
            tc.tile_pool(name="hstp", bufs=2) as hstp,
            tc.tile_pool(name="rope", bufs=2) as rope,
            tc.tile_pool(name="psA", bufs=3, space="PSUM") as psA,
            tc.tile_pool(name="psT", bufs=2, space="PSUM") as psT,
        ):
            # DMA order matters at startup: get hst[0] + wk + rope tables in
            # first so the k-projection (first job) can start ASAP.
            hst_tiles = []
            hst_t0 = hstp.tile([128, KC, 512], BF16, tag="hst", name="hst0")
            for r in range(4):
                nc.sync.dma_start(
                    hst_t0[:, r * 8 : (r + 1) * 8, :],
                    hst_d[0, :, r * 8 : (r + 1) * 8, :],
                )
            hst_tiles.append(hst_t0)
            wk_sb = wqkv.tile([128, KC, D], BF16)
            nc.sync.dma_start(wk_sb[:], wk_d[:])
            cos_sb = const.tile([128, S], F32)
            sin_sb = const.tile([128, S], F32)
            nc.sync.dma_start(cos_sb[:], cos_d[:])
            nc.sync.dma_start(sin_sb[:], sin_d[:])
            wv_sb = wqkv.tile([128, KC, D], BF16)
            nc.sync.dma_start(wv_sb[:], wv_d[:])
            wq_sb = [
                wqkv.tile([128, KC, D], BF16, tag=f"wq{h}", name=f"wq{h}")
                for h in range(NQ)
            ]
            for h in range(NQ):
                nc.sync.dma_start(wq_sb[h][:], wq_d[h])

            identity = const.tile([128, 128], BF16)
            make_identity(nc, identity[:])
            ones = const.tile([128, 1], BF16)
            nc.gpsimd.memset(ones[:], 1.0)
            ones_f = const.tile([128, 1], F32)
            nc.gpsimd.memset(ones_f[:], 1.0)
            # triangular mask for the diagonal 128x128 subtile: rows are k,
            # cols are q; keep q >= k.
            tri = const.tile([128, 128], BF16)
            nc.gpsimd.memset(tri[:], 1.0)
            nc.gpsimd.affine_select(
                out=tri[:],
                in_=tri[:],
                pattern=[[1, 128]],
                compare_op=mybir.AluOpType.is_ge,
                fill=0.0,
                base=0,
                channel_multiplier=-1,
            )

            def rope_evict(ps, dst_tile, qr):
                """dst[0:64]  = x0*cos - x1*sin
                dst[64:128] = x1*cos + x0*sin   (x0=ps[0:64], x1=ps[64:128])"""
                sl = slice(qr * 512, (qr + 1) * 512)
                raw = rope.tile([128, 512], F32, tag="raw")
                nc.vector.tensor_copy(raw[:], ps[:])
                swp = rope.tile([128, 512], F32, tag="swp")
                nc.sync.dma_start(swp[0:64, :], raw[64:128, :])
                nc.sync.dma_start(swp[64:128, :], raw[0:64, :])
                # in-place: raw *= cos, swp *= sin
                nc.vector.tensor_mul(raw[:], raw[:], cos_sb[:, sl])
                nc.vector.tensor_mul(swp[:], swp[:], sin_sb[:, sl])
                nc.vector.tensor_sub(dst_tile[0:64, sl], raw[0:64, :], swp[0:64, :])
                nc.vector.tensor_add(
                    dst_tile[64:128, sl], raw[64:128, :], swp[64:128, :]
                )

            for qr in range(NQR):
                if qr + 1 < NQR:
                    nxt = hstp.tile([128, KC, 512], BF16, tag="hst", name=f"hst{qr+1}")
                    nc.sync.dma_start(nxt[:], hst_d[qr + 1])
                    hst_tiles.append(nxt)
                hst_t = hst_tiles[qr]
                # k and v first (their weights arrive first)
                jobs = [("k", 0), ("v", 0)] + [("q", h) for h in range(NQ)]
                for kind, h in jobs:
                    ps = psA.tile([128, 512], F32)
                    for c in range(KC):
                        if kind == "q":
                            lhsT = wq_sb[h][:, c, :]
                        elif kind == "k":
                            lhsT = wk_sb[:, c, :]
                        else:
                            lhsT = wv_sb[:, c, :]
                        nc.tensor.matmul(
                            ps[:],
                            lhsT,
                            hst_t[:, c, :],
                            start=(c == 0),
                            stop=(c == KC - 1),
                        )
                    if kind == "q":
                        rope_evict(ps, qt_sb[h], qr)
                    elif kind == "k":
                        rope_evict(ps, kt_sb, qr)
                    else:
                        sl = slice(qr * 512, (qr + 1) * 512)
                        nc.vector.tensor_copy(vt_sb[:, sl], ps[:])
                # transpose this qr's v slice into [s, d] chunks
                for kt in range(qr * 4, qr * 4 + 4):
                    pst = psT.tile([128, 128], BF16)
                    nc.tensor.transpose(
                        pst[:], vt_sb[:, kt * 128 : (kt + 1) * 128], identity[:]
                    )
                    nc.vector.tensor_copy(v_sb[:, kt, :], pst[:])

        # ========== Phase B: causal attention (+ interleaved o_proj) ==========
        wo_pool = stack.enter_context(tc.tile_pool(name="wo", bufs=1))
        wo_sb = wo_pool.tile([128, NQ, HID], BF16)
        nc.sync.dma_start(wo_sb[:], wo_d[:])
        ostage = stack.enter_context(tc.tile_pool(name="ostage", bufs=6))

        # k-tiles are processed in PAIRS: one [128, 1024] PSUM scores tile
        # holds two k-tiles' scoresT so one ACT exp instruction covers both
        # (the 352-cycle per-ACTIVATE overhead would otherwise make ACT the
        # phase-B pacer). Diagonal pairs use two column-narrowed exps.
        #
        # Phase B is exp-throughput-paced on ACT, leaving PE bubbles; o_proj
        # (phase C) matmuls for already-completed q-ranges are interleaved
        # into those bubbles as PE filler. PSUM: scores pairs 2x2 banks +
        # o_proj accumulators 2 + attention output 2 = 8 banks.
        with (
            tc.tile_pool(name="psS", bufs=2, space="PSUM") as psS,
            tc.tile_pool(name="psO", bufs=2, space="PSUM") as psO,
        ):
            # ---- o_proj work-unit generator ----
            def c_units():
                for qrC in range(NQR):
                    for st in range(qrC * 4, qrC * 4 + 4):
                        for ho in range(NHO):
                            yield ("alloc", qrC, st, ho)
                            for h in range(NQ):
                                yield ("mm", qrC, st, ho, h)
                            yield ("evict", qrC, st, ho)

            c_state = {"gen": c_units(), "pending": None, "tile": None,
                       "done": set(), "open": None}

            def emit_c(n_mms, qr_done, evict_engine="v"):
                emitted = 0
                while emitted < n_mms:
                    unit = c_state["pending"] or next(c_state["gen"], None)
                    c_state["pending"] = None
                    if unit is None:
                        return False
                    if unit[1] > qr_done:
                        c_state["pending"] = unit
                        return False
                    if unit[0] == "alloc":
                        _, _, st, ho = unit
                        c_state["open"] = (st, ho)
                        c_state["tile"] = psS.tile(
                            [128, 512], F32, tag="c", bufs=2, name=f"c{st}_{ho}"
                        )
                    elif unit[0] == "mm":
                        _, _, st, ho, h = unit
                        nc.tensor.matmul(
                            c_state["tile"][:],
                            attn_sb[h][:, st * 128 : (st + 1) * 128],
                            wo_sb[:, h, ho * 512 : (ho + 1) * 512],
                            start=(h == 0),
                            stop=(h == NQ - 1),
                            skip_group_check=True,
                        )
                        emitted += 1
                    else:
                        _, _, st, ho = unit
                        c_state["done"].add((st, ho))
                        stg = ostage.tile([128, 512], F32, tag="stg")
                        if evict_engine == "v":
                            nc.vector.tensor_copy(stg[:], c_state["tile"][:])
                        else:
                            nc.scalar.copy(stg[:], c_state["tile"][:])
                        nc.sync.dma_start(
                            out_d[
                                st * 128 : (st + 1) * 128,
                                ho * 512 : (ho + 1) * 512,
                            ],
                            stg[:],
                        )
                return True

            for qr in range(NQR):
                n_kt = 4 * (qr + 1)
                n_pair = n_kt // 2
                qsl = slice(qr * 512, (qr + 1) * 512)

                def c0_of(kt, qr=qr):
                    p_idx = kt - 4 * qr
                    return 128 * p_idx if p_idx > 0 else 0

                for h in range(NQ):
                    ps_o = psO.tile([128, 512], F32, tag="o", name=f"o{qr}_{h}")
                    den_acc = den_p.tile(
                        [128, 512], F32, tag="da", name=f"da{qr}_{h}"
                    )
                    pair_tiles = {}

                    def mm_scores_pair(j, qr=qr, h=h, pair_tiles=pair_tiles):
                        ps_s = psS.tile(
                            [128, 1024], F32, tag="s", name=f"s{qr}_{h}_{j}"
                        )
                        for idx in range(2):
                            kt = 2 * j + idx
                            c0 = c0_of(kt)
                            nc.tensor.matmul(
                                ps_s[:, idx * 512 + c0 : (idx + 1) * 512],
                                kt_sb[:, kt * 128 : (kt + 1) * 128],
                                qt_sb[h][:, qr * 512 + c0 : (qr + 1) * 512],
                                start=True,
                                stop=True,
                            )
                        pair_tiles[j] = ps_s

                    mm_scores_pair(0)
                    for j in range(n_pair):
                        if j + 1 < n_pair:
                            mm_scores_pair(j + 1)
                        emit_c(2, qr - 1)
                        ps_s = pair_tiles.pop(j)
                        pt = probs_p.tile(
                            [128, 1024], BF16, tag="pt", name=f"pt{qr}_{h}_{j}"
                        )
                        if 2 * j + 1 < 4 * qr:
                            # fully off-diagonal pair: one batched exp
                            nc.scalar.activation(
                                pt[:],
                                ps_s[:],
                                mybir.ActivationFunctionType.Exp,
                                scale=SCALE,
                            )
                        else:
                            for idx in range(2):
                                kt = 2 * j + idx
                                c0 = c0_of(kt)
                                nc.scalar.activation(
                                    pt[:, idx * 512 + c0 : (idx + 1) * 512],
                                    ps_s[:, idx * 512 + c0 : (idx + 1) * 512],
                                    mybir.ActivationFunctionType.Exp,
                                    scale=SCALE,
                                )
                                # triangular mask on the diagonal subtile
                                nc.vector.tensor_mul(
                                    pt[:, idx * 512 + c0 : idx * 512 + c0 + 128],
                                    pt[:, idx * 512 + c0 : idx * 512 + c0 + 128],
                                    tri[:],
                                )
                        for idx in range(2):
                            kt = 2 * j + idx
                            c0 = c0_of(kt)
                            nc.tensor.matmul(
                                ps_o[:, c0:512],
                                v_sb[:, kt, :],
                                pt[:, idx * 512 + c0 : (idx + 1) * 512],
                                start=(kt == 0),
                                stop=(kt == n_kt - 1),
                                skip_group_check=True,
                            )
                        # denominator: fold the pair halves (bf16) and
                        # accumulate on DVE in f32 — no per-pair PE matmul
                        pts = probs_p.tile(
                            [128, 512], BF16, tag="pts", name=f"pts{qr}_{h}_{j}"
                        )
                        c0a, c0b = c0_of(2 * j), c0_of(2 * j + 1)
                        if c0b > c0a:
                            nc.vector.tensor_copy(
                                pts[:, c0a:c0b], pt[:, c0a:c0b]
                            )
                        nc.vector.tensor_add(
                            pts[:, c0b:512],
                            pt[:, c0b:512],
                            pt[:, 512 + c0b : 1024],
                        )
                        if j == 0:
                            nc.vector.tensor_copy(den_acc[:], pts[:])
                        else:
                            nc.vector.tensor_add(
                                den_acc[:, c0a:512],
                                den_acc[:, c0a:512],
                                pts[:, c0a:512],
                            )
                    emit_c(2, qr - 1)
                    # partition-reduce den_acc with one fp32 matmul into the
                    # (dead) last scores psum tile, then recip + broadcast
                    nc.tensor.matmul(
                        ps_s[0:1, 0:512],
                        ones_f[:],
                        den_acc[:],
                        start=True,
                        stop=True,
                        skip_group_check=True,
                    )
                    recip = den_p.tile(
                        [1, 512], F32, tag="recip", name=f"rc{qr}_{h}"
                    )
                    nc.vector.reciprocal_approx_fast(
                        out=recip[:], in_=ps_s[0:1, 0:512]
                    )
                    bc = bcast_p.tile([128, 512], F32, tag="bc")
                    nc.gpsimd.partition_broadcast(bc[:], recip[:])
                    nc.vector.tensor_mul(attn_sb[h][:, qsl], ps_o[:], bc[:])

            # ---- drain the remaining o_proj work ----
            alt = 0
            while emit_c(4, NQR - 1, evict_engine=("v" if alt % 2 == 0 else "s")):
                alt += 1

    nc.compile()
    return nc


def _get_nc():
    if "nc" not in _CACHE:
        _CACHE["nc"] = _build_nc()
    return _CACHE["nc"]


def _bf16(x):
    return np.ascontiguousarray(x.astype(ml_dtypes.bfloat16))


def _prep_in_maps(hidden_states, sin_table, cos_table, Wq, Wk, Wv, Wo):
    hs0 = np.asarray(hidden_states, np.float32).reshape(S, HID)
    # hst[qr, p, c, s] = hs0[qr*512 + s, c*128 + p]
    hst = _bf16(hs0.reshape(NQR, 512, KC, 128).transpose(0, 3, 2, 1))
    cosT = np.asarray(cos_table, np.float32).T  # [64, S]
    sinT = np.asarray(sin_table, np.float32).T
    cos2 = np.ascontiguousarray(np.concatenate([cosT, cosT], 0))  # [128, S]
    sin2 = np.ascontiguousarray(np.concatenate([sinT, sinT], 0))
    Wq = np.asarray(Wq, np.float32)
    Wk = np.asarray(Wk, np.float32)
    Wv = np.asarray(Wv, np.float32)
    Wo = np.asarray(Wo, np.float32)

    in_maps = []
    for c in range(N_CORES):
        wq_c = Wq[:, c * 512 : (c + 1) * 512]  # 4 q heads
        wk_c = Wk[:, c * 128 : (c + 1) * 128]  # 1 kv head
        wv_c = Wv[:, c * 128 : (c + 1) * 128]
        wo_c = Wo[c * 512 : (c + 1) * 512, :]  # matching rows
        # wq per-head-major: [h, p, c, d] with element Wq_c[c*128+p, h*128+d]
        wq_l = wq_c.reshape(KC, 128, NQ, D).transpose(2, 1, 0, 3)
        in_maps.append(
            {
                "hst": hst,
                "wq": _bf16(wq_l),
                "wk": _bf16(wk_c.reshape(KC, 128, D).swapaxes(0, 1)),
                "wv": _bf16(wv_c.reshape(KC, 128, D).swapaxes(0, 1)),
                "wo": _bf16(wo_c.reshape(NQ, 128, HID).swapaxes(0, 1)),
                "cos2": cos2,
                "sin2": sin2,
            }
        )
    return in_maps


def run(trace=False, **inputs):
    nc = _get_nc()
    in_maps = _prep_in_maps(**inputs)
    res = run_bass_kernel_spmd(
        nc, in_maps, core_ids=list(range(N_CORES)), trace=trace
    )
    partials = np.stack([res.results[c]["out"] for c in range(N_CORES)])
    out = partials.sum(axis=0, dtype=np.float32).reshape(1, S, HID)
    return out, res


def kernel(**inputs):
    out, _ = run(trace=False, **inputs)
    return out


# revision 37
# speedup vs baseline: 1.0076x; 1.0006x over previous
"""Trainium2 Bass kernel for GQA attention layer (B=1, S=2048, H=4096,
32 Q heads / 8 KV heads, head_dim 128, RoPE with arbitrary tables).

Sharding: tensor-parallel over heads across 8 NeuronCores — core c gets
Q heads 4c..4c+3 and KV head c (Wq/Wk/Wv column shards, Wo row shard).
Each core computes its partial o_proj output [2048, 4096]; the host sums
the 8 partials (equivalent of the all-reduce).

Per-core compute (all matmuls bf16 with fp32 PSUM accumulation):
  Phase A: qT/kT/vT = W.T @ hs.T in [d, s] layout (N=512 matmuls),
           RoPE applied via rotate-half partition swap (SBUF-to-SBUF DMA)
           + elementwise DVE ops; v transposed to [s, d] chunks on PE.
  Phase B: flash-style causal attention per (head, q-range, k-tile pair):
           scoresT[k,q] = kT.T @ qT into a [128,1024] PSUM tile covering
           two k-tiles so one ACT exp instruction amortizes the 352-cycle
           ACTIVATE overhead; probsT = exp(scale*scores) (no
           max-subtraction — scores are bounded, fp32 exp is exact
           enough); attn_oT[d,q] += v[k,d].T @ probsT. Diagonal k-tiles
           are column-narrowed to the unmasked range (q >= 128p) plus one
           triangular 128-col mask multiply. The softmax denominator is
           accumulated on the DVE (bf16 pair-folds into an f32
           accumulator), partition-reduced with one fp32 matmul per head,
           then fast-reciprocal + gpsimd partition_broadcast + DVE mul
           normalize the PSUM attention output.
  Phase C: partial o_proj [s, hidden] = attn_oT.T @ Wo_shard. Phase B is
           exp-throughput-paced on ACT, so o_proj matmuls of completed
           q-ranges are interleaved into B's PE bubbles; the remainder
           drains afterwards.
"""

import sys
from contextlib import ExitStack

sys.path.insert(0, "/opt/trn_rl_repo")

import numpy as np
import ml_dtypes

import concourse.bass as bass
import concourse.bacc as bacc
import concourse.mybir as mybir
import concourse.tile as tile
from concourse.bass_utils import run_bass_kernel_spmd
from concourse import bass_isa
from concourse.masks import make_identity

BF16 = mybir.dt.bfloat16
F32 = mybir.dt.float32

N_CORES = 8
S = 2048
HID = 4096
D = 128
NQ = 4  # q heads per core
KC = HID // 128  # 32 hidden-dim chunks
NQR = S // 512  # 4 q ranges of 512
NST = S // 128  # 16 s-tiles of 128
NHO = HID // 512  # 8 output column tiles of 512
SCALE = 1.0 / float(np.sqrt(D))

_CACHE: dict = {}


def _build_nc():
    nc = bacc.Bacc(None, target_bir_lowering=False, debug=False)

    hst_d = nc.dram_tensor("hst", [NQR, 128, KC, 512], BF16, kind="ExternalInput")
    wq_d = nc.dram_tensor("wq", [NQ, 128, KC, D], BF16, kind="ExternalInput")
    wk_d = nc.dram_tensor("wk", [128, KC, D], BF16, kind="ExternalInput")
    wv_d = nc.dram_tensor("wv", [128, KC, D], BF16, kind="ExternalInput")
    wo_d = nc.dram_tensor("wo", [128, NQ, HID], BF16, kind="ExternalInput")
    cos_d = nc.dram_tensor("cos2", [128, S], F32, kind="ExternalInput")
    sin_d = nc.dram_tensor("sin2", [128, S], F32, kind="ExternalInput")
    out_d = nc.dram_tensor("out", [S, HID], F32, kind="ExternalOutput")

    with tile.TileContext(nc) as tc, ExitStack() as stack:
        # ---- pools that live the whole kernel ----
        const = stack.enter_context(tc.tile_pool(name="const", bufs=1))
        act = stack.enter_context(tc.tile_pool(name="act", bufs=1))
        qt_sb = [
            act.tile([128, S], BF16, tag=f"qt{h}", name=f"qt{h}") for h in range(NQ)
        ]
        kt_sb = act.tile([128, S], BF16, tag="kt")
        vt_sb = act.tile([128, S], BF16, tag="vt")
        v_sb = act.tile([128, NST, 128], BF16, tag="v")  # [s,d] chunks per k-tile
        attn_sb = [
            act.tile([128, S], BF16, tag=f"attn{h}", name=f"attn{h}")
            for h in range(NQ)
        ]
        # B-phase SBUF pools are allocated up-front (NOT from space reused
        # from the A-phase pools) — otherwise the first B eviction picks up
        # a WAR dependency on the tail of phase A and the PE/DVE/GPSIMD
        # engines form a multi-microsecond circular stall.
        probs_p = stack.enter_context(tc.tile_pool(name="probs", bufs=3))
        den_p = stack.enter_context(tc.tile_pool(name="den", bufs=2))
        bcast_p = stack.enter_context(tc.tile_pool(name="bcast", bufs=2))

        # ================= Phase A: QKV projections + RoPE =================
        with (
            tc.tile_pool(name="wqkv", bufs=1) as wqkv,
            tc.tile_pool(name="hstp", bufs=2) as hstp,
            tc.tile_pool(name="rope", bufs=2) as rope,
            tc.tile_pool(name="psA", bufs=3, space="PSUM") as psA,
            tc.tile_pool(name="psT", bufs=2, space="PSUM") as psT,
        ):
            # DMA order matters at startup: get hst[0] + wk + rope tables in
            # first so the k-projection (first job) can start ASAP.
            hst_tiles = []
            hst_t0 = hstp.tile([128, KC, 512], BF16, tag="hst", name="hst0")
            for r in range(4):
                nc.sync.dma_start(
                    hst_t0[:, r * 8 : (r + 1) * 8, :],
                    hst_d[0, :, r * 8 : (r + 1) * 8, :],
                )
            hst_tiles.append(hst_t0)
            wk_sb = wqkv.tile([128, KC, D], BF16)
            nc.sync.dma_start(wk_sb[:], wk_d[:])
            cos_sb = const.tile([128, S], F32)
            sin_sb = const.tile([128, S], F32)
            nc.sync.dma_start(cos_sb[:], cos_d[:])
            nc.sync.dma_start(sin_sb[:], sin_d[:])
            wv_sb = wqkv.tile([128, KC, D], BF16)
            nc.sync.dma_start(wv_sb[:], wv_d[:])
            wq_sb = [
                wqkv.tile([128, KC, D], BF16, tag=f"wq{h}", name=f"wq{h}")
                for h in range(NQ)
            ]
            for h in range(NQ):
                nc.sync.dma_start(wq_sb[h][:], wq_d[h])

            identity = const.tile([128, 128], BF16)
            make_identity(nc, identity[:])
            ones = const.tile([128, 1], BF16)
            nc.gpsimd.memset(ones[:], 1.0)
            ones_f = const.tile([128, 1], F32)
            nc.gpsimd.memset(ones_f[:], 1.0)
            # triangular mask for the diagonal 128x128 subtile: rows are k,
            # cols are q; keep q >= k.
            tri = const.tile([128, 128], BF16)
            nc.gpsimd.memset(tri[:], 1.0)
            nc.gpsimd.affine_select(
                out=tri[:],
                in_=tri[:],
                pattern=[[1, 128]],
                compare_op=mybir.AluOpType.is_ge,
                fill=0.0,
                base=0,
                channel_multiplier=-1,
            )

            def rope_evict(ps, dst_tile, qr):
                """dst[0:64]  = x0*cos - x1*sin
                dst[64:128] = x1*cos + x0*sin   (x0=ps[0:64], x1=ps[64:128])"""
                sl = slice(qr * 512, (qr + 1) * 512)
                raw = rope.tile([128, 512], F32, tag="raw")
                nc.vector.tensor_copy(raw[:], ps[:])
                swp = rope.tile([128, 512], F32, tag="swp")
                nc.sync.dma_start(swp[0:64, :], raw[64:128, :])
                nc.sync.dma_start(swp[64:128, :], raw[0:64, :])
                # in-place: raw *= cos, swp *= sin
                nc.vector.tensor_mul(raw[:], raw[:], cos_sb[:, sl])
                nc.vector.tensor_mul(swp[:], swp[:], sin_sb[:, sl])
                nc.vector.tensor_sub(dst_tile[0:64, sl], raw[0:64, :], swp[0:64, :])
                nc.vector.tensor_add(
                    dst_tile[64:128, sl], raw[64:128, :], swp[64:128, :]
                )

            for qr in range(NQR):
                if qr + 1 < NQR:
                    nxt = hstp.tile([128, KC, 512], BF16, tag="hst", name=f"hst{qr+1}")
                    nc.sync.dma_start(nxt[:], hst_d[qr + 1])
                    hst_tiles.append(nxt)
                hst_t = hst_tiles[qr]
                # k and v first (their weights arrive first)
                jobs = [("k", 0), ("v", 0)] + [("q", h) for h in range(NQ)]
                for kind, h in jobs:
                    ps = psA.tile([128, 512], F32)
                    for c in range(KC):
                        if kind == "q":
                            lhsT = wq_sb[h][:, c, :]
                        elif kind == "k":
                            lhsT = wk_sb[:, c, :]
                        else:
                            lhsT = wv_sb[:, c, :]
                        nc.tensor.matmul(
                            ps[:],
                            lhsT,
                            hst_t[:, c, :],
                            start=(c == 0),
                            stop=(c == KC - 1),
                        )
                    if kind == "q":
                        rope_evict(ps, qt_sb[h], qr)
                    elif kind == "k":
                        rope_evict(ps, kt_sb, qr)
                    else:
                        sl = slice(qr * 512, (qr + 1) * 512)
                        nc.vector.tensor_copy(vt_sb[:, sl], ps[:])
                # transpose this qr's v slice into [s, d] chunks
                for kt in range(qr * 4, qr * 4 + 4):
                    pst = psT.tile([128, 128], BF16)
                    nc.tensor.transpose(
                        pst[:], vt_sb[:, kt * 128 : (kt + 1) * 128], identity[:]
                    )
                    nc.vector.tensor_copy(v_sb[:, kt, :], pst[:])

        # ========== Phase B: causal attention (+ interleaved o_proj) ==========
        wo_pool = stack.enter_context(tc.tile_pool(name="wo", bufs=1))
        wo_sb = wo_pool.tile([128, NQ, HID], BF16)
        nc.sync.dma_start(wo_sb[:], wo_d[:])
        ostage = stack.enter_context(tc.tile_pool(name="ostage", bufs=6))

        # k-tiles are processed in PAIRS: one [128, 1024] PSUM scores tile
        # holds two k-tiles' scoresT so one ACT exp instruction covers both
        # (the 352-cycle per-ACTIVATE overhead would otherwise make ACT the
        # phase-B pacer). Diagonal pairs use two column-narrowed exps.
        #
        # Phase B is exp-throughput-paced on ACT, leaving PE bubbles; o_proj
        # (phase C) matmuls for already-completed q-ranges are interleaved
        # into those bubbles as PE filler. PSUM: scores pairs 2x2 banks +
        # o_proj accumulators 2 + attention output 2 = 8 banks.
        with (
            tc.tile_pool(name="psS", bufs=2, space="PSUM") as psS,
            tc.tile_pool(name="psO", bufs=2, space="PSUM") as psO,
        ):
            # ---- o_proj work-unit generator ----
            def c_units():
                for qrC in range(NQR):
                    for st in range(qrC * 4, qrC * 4 + 4):
                        for ho in range(NHO):
                            yield ("alloc", qrC, st, ho)
                            for h in range(NQ):
                                yield ("mm", qrC, st, ho, h)
                            yield ("evict", qrC, st, ho)

            c_state = {"gen": c_units(), "pending": None, "tile": None,
                       "done": set(), "open": None}

            def emit_c(n_mms, qr_done, evict_engine="v"):
                emitted = 0
                while emitted < n_mms:
                    unit = c_state["pending"] or next(c_state["gen"], None)
                    c_state["pending"] = None
                    if unit is None:
                        return False
                    if unit[1] > qr_done:
                        c_state["pending"] = unit
                        return False
                    if unit[0] == "alloc":
                        _, _, st, ho = unit
                        c_state["open"] = (st, ho)
                        c_state["tile"] = psS.tile(
                            [128, 512], F32, tag="c", bufs=2, name=f"c{st}_{ho}"
                        )
                    elif unit[0] == "mm":
                        _, _, st, ho, h = unit
                        nc.tensor.matmul(
                            c_state["tile"][:],
                            attn_sb[h][:, st * 128 : (st + 1) * 128],
                            wo_sb[:, h, ho * 512 : (ho + 1) * 512],
                            start=(h == 0),
                            stop=(h == NQ - 1),
                            skip_group_check=True,
                        )
                        emitted += 1
                    else:
                        _, _, st, ho = unit
                        c_state["done"].add((st, ho))
                        stg = ostage.tile([128, 512], F32, tag="stg")
                        if evict_engine == "v":
                            nc.vector.tensor_copy(stg[:], c_state["tile"][:])
                        else:
                            nc.scalar.copy(stg[:], c_state["tile"][:])
                        nc.sync.dma_start(
                            out_d[
                                st * 128 : (st + 1) * 128,
                                ho * 512 : (ho + 1) * 512,
                            ],
                            stg[:],
                        )
                return True

            for qr in range(NQR):
                n_kt = 4 * (qr + 1)
                n_pair = n_kt // 2
                qsl = slice(qr * 512, (qr + 1) * 512)

                def c0_of(kt, qr=qr):
                    p_idx = kt - 4 * qr
                    return 128 * p_idx if p_idx > 0 else 0

                for h in range(NQ):
                    ps_o = psO.tile([128, 512], F32, tag="o", name=f"o{qr}_{h}")
                    den_acc = den_p.tile(
                        [128, 512], F32, tag="da", name=f"da{qr}_{h}"
                    )
                    pair_tiles = {}

                    def mm_scores_pair(j, qr=qr, h=h, pair_tiles=pair_tiles):
                        ps_s = psS.tile(
                            [128, 1024], F32, tag="s", name=f"s{qr}_{h}_{j}"
                        )
                        for idx in range(2):
                            kt = 2 * j + idx
                            c0 = c0_of(kt)
                            nc.tensor.matmul(
                                ps_s[:, idx * 512 + c0 : (idx + 1) * 512],
                                kt_sb[:, kt * 128 : (kt + 1) * 128],
                                qt_sb[h][:, qr * 512 + c0 : (qr + 1) * 512],
                                start=True,
                                stop=True,
                            )
                        pair_tiles[j] = ps_s

                    mm_scores_pair(0)
                    for j in range(n_pair):
                        if j + 1 < n_pair:
                            mm_scores_pair(j + 1)
                        emit_c(2, qr - 1)
                        ps_s = pair_tiles.pop(j)
                        pt = probs_p.tile(
                            [128, 1024], BF16, tag="pt", name=f"pt{qr}_{h}_{j}"
                        )
                        if 2 * j + 1 < 4 * qr:
                            # fully off-diagonal pair: one batched exp
                            nc.scalar.activation(
                                pt[:],
                                ps_s[:],
                                mybir.ActivationFunctionType.Exp,
                                scale=SCALE,
                            )
                        else:
                            for idx in range(2):
                                kt = 2 * j + idx
                                c0 = c0_of(kt)
                                nc.scalar.activation(
                                    pt[:, idx * 512 + c0 : (idx + 1) * 512],
                                    ps_s[:, idx * 512 + c0 : (idx + 1) * 512],
                                    mybir.ActivationFunctionType.Exp,
                                    scale=SCALE,
                                )
                                # triangular mask on the diagonal subtile
                                nc.vector.tensor_mul(
                                    pt[:, idx * 512 + c0 : idx * 512 + c0 + 128],
                                    pt[:, idx * 512 + c0 : idx * 512 + c0 + 128],
                                    tri[:],
                                )
                        for idx in range(2):
                            kt = 2 * j + idx
                            c0 = c0_of(kt)
                            nc.tensor.matmul(
                                ps_o[:, c0:512],
                                v_sb[:, kt, :],
                                pt[:, idx * 512 + c0 : (idx + 1) * 512],
                                start=(kt == 0),
                                stop=(kt == n_kt - 1),
                                skip_group_check=True,
                            )
                        # denominator: fold the pair halves (bf16) and
                        # accumulate on DVE in f32 — no per-pair PE matmul
                        pts = probs_p.tile(
                            [128, 512], BF16, tag="pts", name=f"pts{qr}_{h}_{j}"
                        )
                        c0a, c0b = c0_of(2 * j), c0_of(2 * j + 1)
                        if c0b > c0a:
                            nc.vector.tensor_copy(
                                pts[:, c0a:c0b], pt[:, c0a:c0b]
                            )
                        nc.vector.tensor_add(
                            pts[:, c0b:512],
                            pt[:, c0b:512],
                            pt[:, 512 + c0b : 1024],
                        )
                        if j == 0:
                            nc.vector.tensor_copy(den_acc[:], pts[:])
                        else:
                            nc.vector.tensor_add(
                                den_acc[:, c0a:512],
                                den_acc[:, c0a:512],
                                pts[:, c0a:512],
                            )
                    emit_c(2, qr - 1)
                    # partition-reduce den_acc with one fp32 matmul into the
                    # (dead) last scores psum tile, then recip + broadcast
                    nc.tensor.matmul(
                        ps_s[0:1, 0:512],
                        ones_f[:],
                        den_acc[:],
                        start=True,
                        stop=True,
                        skip_group_check=True,
                    )
                    recip = den_p.tile(
                        [1, 512], F32, tag="recip", name=f"rc{qr}_{h}"
                    )
                    nc.vector.reciprocal_approx_fast(
                        out=recip[:], in_=ps_s[0:1, 0:512]
                    )
                    bc = bcast_p.tile([128, 512], F32, tag="bc")
                    nc.gpsimd.partition_broadcast(bc[:], recip[:])
                    nc.vector.tensor_mul(attn_sb[h][:, qsl], ps_o[:], bc[:])

            # ---- drain the remaining o_proj work ----
            alt = 0
            while emit_c(4, NQR - 1, evict_engine=("v" if alt % 2 == 0 else "s")):
                alt += 1

    nc.compile()
    return nc


def _get_nc():
    if "nc" not in _CACHE:
        _CACHE["nc"] = _build_nc()
    return _CACHE["nc"]


def _bf16(x):
    return np.ascontiguousarray(x.astype(ml_dtypes.bfloat16))


def _prep_in_maps(hidden_states, sin_table, cos_table, Wq, Wk, Wv, Wo):
    hs0 = np.asarray(hidden_states, np.float32).reshape(S, HID)
    # hst[qr, p, c, s] = hs0[qr*512 + s, c*128 + p]
    hst = _bf16(hs0.reshape(NQR, 512, KC, 128).transpose(0, 3, 2, 1))
    cosT = np.asarray(cos_table, np.float32).T  # [64, S]
    sinT = np.asarray(sin_table, np.float32).T
    cos2 = np.ascontiguousarray(np.concatenate([cosT, cosT], 0))  # [128, S]
    sin2 = np.ascontiguousarray(np.concatenate([sinT, sinT], 0))
    Wq = np.asarray(Wq, np.float32)
    Wk = np.asarray(Wk, np.float32)
    Wv = np.asarray(Wv, np.float32)
    Wo = np.asarray(Wo, np.float32)

    in_maps = []
    for c in range(N_CORES):
        wq_c = Wq[:, c * 512 : (c + 1) * 512]  # 4 q heads
        wk_c = Wk[:, c * 128 : (c + 1) * 128]  # 1 kv head
        wv_c = Wv[:, c * 128 : (c + 1) * 128]
        wo_c = Wo[c * 512 : (c + 1) * 512, :]  # matching rows
        # wq per-head-major: [h, p, c, d] with element Wq_c[c*128+p, h*128+d]
        wq_l = wq_c.reshape(KC, 128, NQ, D).transpose(2, 1, 0, 3)
        in_maps.append(
            {
                "hst": hst,
                "wq": _bf16(wq_l),
                "wk": _bf16(wk_c.reshape(KC, 128, D).swapaxes(0, 1)),
                "wv": _bf16(wv_c.reshape(KC, 128, D).swapaxes(0, 1)),
                "wo": _bf16(wo_c.reshape(NQ, 128, HID).swapaxes(0, 1)),
                "cos2": cos2,
                "sin2": sin2,
            }
        )
    return in_maps


def run(trace=False, **inputs):
    nc = _get_nc()
    in_maps = _prep_in_maps(**inputs)
    res = run_bass_kernel_spmd(
        nc, in_maps, core_ids=list(range(N_CORES)), trace=trace
    )
    partials = np.stack([res.results[c]["out"] for c in range(N_CORES)])
    out = partials.sum(axis=0, dtype=np.float32).reshape(1, S, HID)
    return out, res


def kernel(**inputs):
    out, _ = run(trace=False, **inputs)
    return out


# revision 38
# speedup vs baseline: 1.0189x; 1.0112x over previous
"""Trainium2 Bass kernel for GQA attention layer (B=1, S=2048, H=4096,
32 Q heads / 8 KV heads, head_dim 128, RoPE with arbitrary tables).

Sharding: tensor-parallel over heads across 8 NeuronCores — core c gets
Q heads 4c..4c+3 and KV head c (Wq/Wk/Wv column shards, Wo row shard).
Each core computes its partial o_proj output [2048, 4096]; the host sums
the 8 partials (equivalent of the all-reduce).

Per-core compute (all matmuls bf16 with fp32 PSUM accumulation):
  Phase A: qT/kT/vT = W.T @ hs.T in [d, s] layout (N=512 matmuls),
           RoPE applied via rotate-half partition swap (SBUF-to-SBUF DMA)
           + elementwise DVE ops; v transposed to [s, d] chunks on PE.
  Phase B: flash-style causal attention per (head, q-range, k-tile pair):
           scoresT[k,q] = kT.T @ qT into a [128,1024] PSUM tile covering
           two k-tiles so one ACT exp instruction amortizes the 352-cycle
           ACTIVATE overhead; probsT = exp(scale*scores) (no
           max-subtraction — scores are bounded, fp32 exp is exact
           enough); attn_oT[d,q] += v[k,d].T @ probsT. Diagonal k-tiles
           are column-narrowed to the unmasked range (q >= 128p) plus one
           triangular 128-col mask multiply. The softmax denominator is
           accumulated on the DVE (bf16 pair-folds into an f32
           accumulator), partition-reduced with one fp32 matmul per head,
           then fast-reciprocal + gpsimd partition_broadcast + DVE mul
           normalize the PSUM attention output.
  Phase C: partial o_proj [s, hidden] = attn_oT.T @ Wo_shard. Phase B is
           exp-throughput-paced on ACT, so o_proj matmuls of completed
           q-ranges are interleaved into B's PE bubbles; the remainder
           drains afterwards.
"""

import sys
from contextlib import ExitStack

sys.path.insert(0, "/opt/trn_rl_repo")

import numpy as np
import ml_dtypes

import concourse.bass as bass
import concourse.bacc as bacc
import concourse.mybir as mybir
import concourse.tile as tile
from concourse.bass_utils import run_bass_kernel_spmd
from concourse import bass_isa
from concourse.masks import make_identity

BF16 = mybir.dt.bfloat16
F32 = mybir.dt.float32

N_CORES = 8
S = 2048
HID = 4096
D = 128
NQ = 4  # q heads per core
KC = HID // 128  # 32 hidden-dim chunks
NQR = S // 512  # 4 q ranges of 512
NST = S // 128  # 16 s-tiles of 128
NHO = HID // 512  # 8 output column tiles of 512
SCALE = 1.0 / float(np.sqrt(D))

_CACHE: dict = {}


def _build_nc():
    nc = bacc.Bacc(None, target_bir_lowering=False, debug=False)

    hst_d = nc.dram_tensor("hst", [NQR, 128, KC, 512], BF16, kind="ExternalInput")
    wq_d = nc.dram_tensor("wq", [NQ, 128, KC, D], BF16, kind="ExternalInput")
    wk_d = nc.dram_tensor("wk", [128, KC, D], BF16, kind="ExternalInput")
    wv_d = nc.dram_tensor("wv", [128, KC, D], BF16, kind="ExternalInput")
    wo_d = nc.dram_tensor("wo", [128, NQ, HID], BF16, kind="ExternalInput")
    cos_d = nc.dram_tensor("cos2", [128, S], F32, kind="ExternalInput")
    sin_d = nc.dram_tensor("sin2", [128, S], F32, kind="ExternalInput")
    out_d = nc.dram_tensor("out", [S, HID], F32, kind="ExternalOutput")

    with tile.TileContext(nc) as tc, ExitStack() as stack:
        # ---- pools that live the whole kernel ----
        const = stack.enter_context(tc.tile_pool(name="const", bufs=1))
        act = stack.enter_context(tc.tile_pool(name="act", bufs=1))
        qt_sb = [
            act.tile([128, S], BF16, tag=f"qt{h}", name=f"qt{h}") for h in range(NQ)
        ]
        kt_sb = act.tile([128, S], BF16, tag="kt")
        vt_sb = act.tile([128, S], BF16, tag="vt")
        v_sb = act.tile([128, NST, 128], BF16, tag="v")  # [s,d] chunks per k-tile
        attn_sb = [
            act.tile([128, S], BF16, tag=f"attn{h}", name=f"attn{h}")
            for h in range(NQ)
        ]
        # B-phase SBUF pools are allocated up-front (NOT from space reused
        # from the A-phase pools) — otherwise the first B eviction picks up
        # a WAR dependency on the tail of phase A and the PE/DVE/GPSIMD
        # engines form a multi-microsecond circular stall.
        probs_p = stack.enter_context(tc.tile_pool(name="probs", bufs=3))
        den_p = stack.enter_context(tc.tile_pool(name="den", bufs=2))
        bcast_p = stack.enter_context(tc.tile_pool(name="bcast", bufs=2))

        # ================= Phase A: QKV projections + RoPE =================
        with (
            tc.tile_pool(name="wqkv", bufs=1) as wqkv,
            tc.tile_pool(name="hstp", bufs=2) as hstp,
            tc.tile_pool(name="rope", bufs=2) as rope,
            tc.tile_pool(name="psA", bufs=3, space="PSUM") as psA,
            tc.tile_pool(name="psT", bufs=2, space="PSUM") as psT,
        ):
            # DMA order matters at startup: get hst[0] + wk + rope tables in
            # first so the k-projection (first job) can start ASAP.
            hst_tiles = []
            # wk (1MB) first: the k-projection's c=0 matmul needs it plus
            # only the first hst region, so PE starts ~8us earlier than
            # with wk queued behind the full 4MB hst[0] transfer.
            wk_sb = wqkv.tile([128, KC, D], BF16)
            nc.sync.dma_start(wk_sb[:], wk_d[:])
            hst_t0 = hstp.tile([128, KC, 512], BF16, tag="hst", name="hst0")
            for r in range(4):
                nc.sync.dma_start(
                    hst_t0[:, r * 8 : (r + 1) * 8, :],
                    hst_d[0, :, r * 8 : (r + 1) * 8, :],
                )
            hst_tiles.append(hst_t0)
            wv_sb = wqkv.tile([128, KC, D], BF16)
            nc.sync.dma_start(wv_sb[:], wv_d[:])
            cos_sb = const.tile([128, S], F32)
            sin_sb = const.tile([128, S], F32)
            nc.sync.dma_start(cos_sb[:], cos_d[:])
            nc.sync.dma_start(sin_sb[:], sin_d[:])
            wq_sb = [
                wqkv.tile([128, KC, D], BF16, tag=f"wq{h}", name=f"wq{h}")
                for h in range(NQ)
            ]
            for h in range(NQ):
                nc.sync.dma_start(wq_sb[h][:], wq_d[h])

            identity = const.tile([128, 128], BF16)
            make_identity(nc, identity[:])
            ones = const.tile([128, 1], BF16)
            nc.gpsimd.memset(ones[:], 1.0)
            ones_f = const.tile([128, 1], F32)
            nc.gpsimd.memset(ones_f[:], 1.0)
            # triangular mask for the diagonal 128x128 subtile: rows are k,
            # cols are q; keep q >= k.
            tri = const.tile([128, 128], BF16)
            nc.gpsimd.memset(tri[:], 1.0)
            nc.gpsimd.affine_select(
                out=tri[:],
                in_=tri[:],
                pattern=[[1, 128]],
                compare_op=mybir.AluOpType.is_ge,
                fill=0.0,
                base=0,
                channel_multiplier=-1,
            )

            def rope_evict(ps, dst_tile, qr):
                """dst[0:64]  = x0*cos - x1*sin
                dst[64:128] = x1*cos + x0*sin   (x0=ps[0:64], x1=ps[64:128])"""
                sl = slice(qr * 512, (qr + 1) * 512)
                raw = rope.tile([128, 512], F32, tag="raw")
                nc.vector.tensor_copy(raw[:], ps[:])
                swp = rope.tile([128, 512], F32, tag="swp")
                nc.sync.dma_start(swp[0:64, :], raw[64:128, :])
                nc.sync.dma_start(swp[64:128, :], raw[0:64, :])
                # in-place: raw *= cos, swp *= sin
                nc.vector.tensor_mul(raw[:], raw[:], cos_sb[:, sl])
                nc.vector.tensor_mul(swp[:], swp[:], sin_sb[:, sl])
                nc.vector.tensor_sub(dst_tile[0:64, sl], raw[0:64, :], swp[0:64, :])
                nc.vector.tensor_add(
                    dst_tile[64:128, sl], raw[64:128, :], swp[64:128, :]
                )

            for qr in range(NQR):
                if qr + 1 < NQR:
                    nxt = hstp.tile([128, KC, 512], BF16, tag="hst", name=f"hst{qr+1}")
                    nc.sync.dma_start(nxt[:], hst_d[qr + 1])
                    hst_tiles.append(nxt)
                hst_t = hst_tiles[qr]
                # k and v first (their weights arrive first)
                jobs = [("k", 0), ("v", 0)] + [("q", h) for h in range(NQ)]
                for kind, h in jobs:
                    ps = psA.tile([128, 512], F32)
                    for c in range(KC):
                        if kind == "q":
                            lhsT = wq_sb[h][:, c, :]
                        elif kind == "k":
                            lhsT = wk_sb[:, c, :]
                        else:
                            lhsT = wv_sb[:, c, :]
                        nc.tensor.matmul(
                            ps[:],
                            lhsT,
                            hst_t[:, c, :],
                            start=(c == 0),
                            stop=(c == KC - 1),
                        )
                    if kind == "q":
                        rope_evict(ps, qt_sb[h], qr)
                    elif kind == "k":
                        rope_evict(ps, kt_sb, qr)
                    else:
                        sl = slice(qr * 512, (qr + 1) * 512)
                        nc.vector.tensor_copy(vt_sb[:, sl], ps[:])
                # transpose this qr's v slice into [s, d] chunks
                for kt in range(qr * 4, qr * 4 + 4):
                    pst = psT.tile([128, 128], BF16)
                    nc.tensor.transpose(
                        pst[:], vt_sb[:, kt * 128 : (kt + 1) * 128], identity[:]
                    )
                    nc.vector.tensor_copy(v_sb[:, kt, :], pst[:])

        # ========== Phase B: causal attention (+ interleaved o_proj) ==========
        wo_pool = stack.enter_context(tc.tile_pool(name="wo", bufs=1))
        wo_sb = wo_pool.tile([128, NQ, HID], BF16)
        nc.sync.dma_start(wo_sb[:], wo_d[:])
        ostage = stack.enter_context(tc.tile_pool(name="ostage", bufs=6))

        # k-tiles are processed in PAIRS: one [128, 1024] PSUM scores tile
        # holds two k-tiles' scoresT so one ACT exp instruction covers both
        # (the 352-cycle per-ACTIVATE overhead would otherwise make ACT the
        # phase-B pacer). Diagonal pairs use two column-narrowed exps.
        #
        # Phase B is exp-throughput-paced on ACT, leaving PE bubbles; o_proj
        # (phase C) matmuls for already-completed q-ranges are interleaved
        # into those bubbles as PE filler. PSUM: scores pairs 2x2 banks +
        # o_proj accumulators 2 + attention output 2 = 8 banks.
        with (
            tc.tile_pool(name="psS", bufs=2, space="PSUM") as psS,
            tc.tile_pool(name="psO", bufs=2, space="PSUM") as psO,
        ):
            # ---- o_proj work-unit generator ----
            def c_units():
                for qrC in range(NQR):
                    for st in range(qrC * 4, qrC * 4 + 4):
                        for ho in range(NHO):
                            yield ("alloc", qrC, st, ho)
                            for h in range(NQ):
                                yield ("mm", qrC, st, ho, h)
                            yield ("evict", qrC, st, ho)

            c_state = {"gen": c_units(), "pending": None, "tile": None,
                       "done": set(), "open": None}

            def emit_c(n_mms, qr_done, evict_engine="v"):
                emitted = 0
                while emitted < n_mms:
                    unit = c_state["pending"] or next(c_state["gen"], None)
                    c_state["pending"] = None
                    if unit is None:
                        return False
                    if unit[1] > qr_done:
                        c_state["pending"] = unit
                        return False
                    if unit[0] == "alloc":
                        _, _, st, ho = unit
                        c_state["open"] = (st, ho)
                        c_state["tile"] = psS.tile(
                            [128, 512], F32, tag="c", bufs=2, name=f"c{st}_{ho}"
                        )
                    elif unit[0] == "mm":
                        _, _, st, ho, h = unit
                        nc.tensor.matmul(
                            c_state["tile"][:],
                            attn_sb[h][:, st * 128 : (st + 1) * 128],
                            wo_sb[:, h, ho * 512 : (ho + 1) * 512],
                            start=(h == 0),
                            stop=(h == NQ - 1),
                            skip_group_check=True,
                        )
                        emitted += 1
                    else:
                        _, _, st, ho = unit
                        c_state["done"].add((st, ho))
                        stg = ostage.tile([128, 512], F32, tag="stg")
                        if evict_engine == "v":
                            nc.vector.tensor_copy(stg[:], c_state["tile"][:])
                        else:
                            nc.scalar.copy(stg[:], c_state["tile"][:])
                        nc.sync.dma_start(
                            out_d[
                                st * 128 : (st + 1) * 128,
                                ho * 512 : (ho + 1) * 512,
                            ],
                            stg[:],
                        )
                return True

            for qr in range(NQR):
                n_kt = 4 * (qr + 1)
                n_pair = n_kt // 2
                qsl = slice(qr * 512, (qr + 1) * 512)

                def c0_of(kt, qr=qr):
                    p_idx = kt - 4 * qr
                    return 128 * p_idx if p_idx > 0 else 0

                for h in range(NQ):
                    ps_o = psO.tile([128, 512], F32, tag="o", name=f"o{qr}_{h}")
                    den_acc = den_p.tile(
                        [128, 512], F32, tag="da", name=f"da{qr}_{h}"
                    )
                    pair_tiles = {}

                    def mm_scores_pair(j, qr=qr, h=h, pair_tiles=pair_tiles):
                        ps_s = psS.tile(
                            [128, 1024], F32, tag="s", name=f"s{qr}_{h}_{j}"
                        )
                        for idx in range(2):
                            kt = 2 * j + idx
                            c0 = c0_of(kt)
                            nc.tensor.matmul(
                                ps_s[:, idx * 512 + c0 : (idx + 1) * 512],
                                kt_sb[:, kt * 128 : (kt + 1) * 128],
                                qt_sb[h][:, qr * 512 + c0 : (qr + 1) * 512],
                                start=True,
                                stop=True,
                            )
                        pair_tiles[j] = ps_s

                    mm_scores_pair(0)
                    for j in range(n_pair):
                        if j + 1 < n_pair:
                            mm_scores_pair(j + 1)
                        emit_c(2, qr - 1)
                        ps_s = pair_tiles.pop(j)
                        pt = probs_p.tile(
                            [128, 1024], BF16, tag="pt", name=f"pt{qr}_{h}_{j}"
                        )
                        if 2 * j + 1 < 4 * qr:
                            # fully off-diagonal pair: one batched exp
                            nc.scalar.activation(
                                pt[:],
                                ps_s[:],
                                mybir.ActivationFunctionType.Exp,
                                scale=SCALE,
                            )
                        else:
                            for idx in range(2):
                                kt = 2 * j + idx
                                c0 = c0_of(kt)
                                nc.scalar.activation(
                                    pt[:, idx * 512 + c0 : (idx + 1) * 512],
                                    ps_s[:, idx * 512 + c0 : (idx + 1) * 512],
                                    mybir.ActivationFunctionType.Exp,
                                    scale=SCALE,
                                )
                                # triangular mask on the diagonal subtile
                                nc.vector.tensor_mul(
                                    pt[:, idx * 512 + c0 : idx * 512 + c0 + 128],
                                    pt[:, idx * 512 + c0 : idx * 512 + c0 + 128],
                                    tri[:],
                                )
                        for idx in range(2):
                            kt = 2 * j + idx
                            c0 = c0_of(kt)
                            nc.tensor.matmul(
                                ps_o[:, c0:512],
                                v_sb[:, kt, :],
                                pt[:, idx * 512 + c0 : (idx + 1) * 512],
                                start=(kt == 0),
                                stop=(kt == n_kt - 1),
                                skip_group_check=True,
                            )
                        # denominator: fold the pair halves (bf16) and
                        # accumulate on DVE in f32 — no per-pair PE matmul
                        pts = probs_p.tile(
                            [128, 512], BF16, tag="pts", name=f"pts{qr}_{h}_{j}"
                        )
                        c0a, c0b = c0_of(2 * j), c0_of(2 * j + 1)
                        if c0b > c0a:
                            nc.vector.tensor_copy(
                                pts[:, c0a:c0b], pt[:, c0a:c0b]
                            )
                        nc.vector.tensor_add(
                            pts[:, c0b:512],
                            pt[:, c0b:512],
                            pt[:, 512 + c0b : 1024],
                        )
                        if j == 0:
                            nc.vector.tensor_copy(den_acc[:], pts[:])
                        else:
                            nc.vector.tensor_add(
                                den_acc[:, c0a:512],
                                den_acc[:, c0a:512],
                                pts[:, c0a:512],
                            )
                    emit_c(2, qr - 1)
                    # partition-reduce den_acc with one fp32 matmul into the
                    # (dead) last scores psum tile, then recip + broadcast
                    nc.tensor.matmul(
                        ps_s[0:1, 0:512],
                        ones_f[:],
                        den_acc[:],
                        start=True,
                        stop=True,
                        skip_group_check=True,
                    )
                    recip = den_p.tile(
                        [1, 512], F32, tag="recip", name=f"rc{qr}_{h}"
                    )
                    nc.vector.reciprocal_approx_fast(
                        out=recip[:], in_=ps_s[0:1, 0:512]
                    )
                    bc = bcast_p.tile([128, 512], F32, tag="bc")
                    nc.gpsimd.partition_broadcast(bc[:], recip[:])
                    nc.vector.tensor_mul(attn_sb[h][:, qsl], ps_o[:], bc[:])

            # ---- drain the remaining o_proj work ----
            alt = 0
            while emit_c(4, NQR - 1, evict_engine=("v" if alt % 2 == 0 else "s")):
                alt += 1

    nc.compile()
    return nc


def _get_nc():
    if "nc" not in _CACHE:
        _CACHE["nc"] = _build_nc()
    return _CACHE["nc"]


def _bf16(x):
    return np.ascontiguousarray(x.astype(ml_dtypes.bfloat16))


def _prep_in_maps(hidden_states, sin_table, cos_table, Wq, Wk, Wv, Wo):
    hs0 = np.asarray(hidden_states, np.float32).reshape(S, HID)
    # hst[qr, p, c, s] = hs0[qr*512 + s, c*128 + p]
    hst = _bf16(hs0.reshape(NQR, 512, KC, 128).transpose(0, 3, 2, 1))
    cosT = np.asarray(cos_table, np.float32).T  # [64, S]
    sinT = np.asarray(sin_table, np.float32).T
    cos2 = np.ascontiguousarray(np.concatenate([cosT, cosT], 0))  # [128, S]
    sin2 = np.ascontiguousarray(np.concatenate([sinT, sinT], 0))
    Wq = np.asarray(Wq, np.float32)
    Wk = np.asarray(Wk, np.float32)
    Wv = np.asarray(Wv, np.float32)
    Wo = np.asarray(Wo, np.float32)

    in_maps = []
    for c in range(N_CORES):
        wq_c = Wq[:, c * 512 : (c + 1) * 512]  # 4 q heads
        wk_c = Wk[:, c * 128 : (c + 1) * 128]  # 1 kv head
        wv_c = Wv[:, c * 128 : (c + 1) * 128]
        wo_c = Wo[c * 512 : (c + 1) * 512, :]  # matching rows
        # wq per-head-major: [h, p, c, d] with element Wq_c[c*128+p, h*128+d]
        wq_l = wq_c.reshape(KC, 128, NQ, D).transpose(2, 1, 0, 3)
        in_maps.append(
            {
                "hst": hst,
                "wq": _bf16(wq_l),
                "wk": _bf16(wk_c.reshape(KC, 128, D).swapaxes(0, 1)),
                "wv": _bf16(wv_c.reshape(KC, 128, D).swapaxes(0, 1)),
                "wo": _bf16(wo_c.reshape(NQ, 128, HID).swapaxes(0, 1)),
                "cos2": cos2,
                "sin2": sin2,
            }
        )
    return in_maps


def run(trace=False, **inputs):
    nc = _get_nc()
    in_maps = _prep_in_maps(**inputs)
    res = run_bass_kernel_spmd(
        nc, in_maps, core_ids=list(range(N_CORES)), trace=trace
    )
    partials = np.stack([res.results[c]["out"] for c in range(N_CORES)])
    out = partials.sum(axis=0, dtype=np.float32).reshape(1, S, HID)
    return out, res


def kernel(**inputs):
    out, _ = run(trace=False, **inputs)
    return out


# revision 39
# speedup vs baseline: 1.0399x; 1.0206x over previous
"""Trainium2 Bass kernel for GQA attention layer (B=1, S=2048, H=4096,
32 Q heads / 8 KV heads, head_dim 128, RoPE with arbitrary tables).

Sharding: tensor-parallel over heads across 8 NeuronCores — core c gets
Q heads 4c..4c+3 and KV head c (Wq/Wk/Wv column shards, Wo row shard).
Each core computes its partial o_proj output [2048, 4096]; the host sums
the 8 partials (equivalent of the all-reduce).

Per-core compute (all matmuls bf16 with fp32 PSUM accumulation):
  Phase A: qT/kT/vT = W.T @ hs.T in [d, s] layout (N=512 matmuls),
           RoPE applied via rotate-half partition swap (SBUF-to-SBUF DMA)
           + elementwise DVE ops; v transposed to [s, d] chunks on PE.
  Phase B: flash-style causal attention per (head, q-range, k-tile pair):
           scoresT[k,q] = kT.T @ qT into a [128,1024] PSUM tile covering
           two k-tiles so one ACT exp instruction amortizes the 352-cycle
           ACTIVATE overhead; probsT = exp(scale*scores) (no
           max-subtraction — scores are bounded, fp32 exp is exact
           enough); attn_oT[d,q] += v[k,d].T @ probsT. Diagonal k-tiles
           are column-narrowed to the unmasked range (q >= 128p) plus one
           triangular 128-col mask multiply. The softmax denominator is
           accumulated on the DVE (bf16 pair-folds into an f32
           accumulator), partition-reduced with one fp32 matmul per head,
           then fast-reciprocal + gpsimd partition_broadcast + DVE mul
           normalize the PSUM attention output.
  Phase C: partial o_proj [s, hidden] = attn_oT.T @ Wo_shard. Phase B is
           exp-throughput-paced on ACT, so o_proj matmuls of completed
           q-ranges are interleaved into B's PE bubbles; the remainder
           drains afterwards.
"""

import sys
from contextlib import ExitStack

sys.path.insert(0, "/opt/trn_rl_repo")

import numpy as np
import ml_dtypes

import concourse.bass as bass
import concourse.bacc as bacc
import concourse.mybir as mybir
import concourse.tile as tile
from concourse.bass_utils import run_bass_kernel_spmd
from concourse import bass_isa
from concourse.masks import make_identity

BF16 = mybir.dt.bfloat16
F32 = mybir.dt.float32

N_CORES = 8
S = 2048
HID = 4096
D = 128
NQ = 4  # q heads per core
KC = HID // 128  # 32 hidden-dim chunks
NQR = S // 512  # 4 q ranges of 512
NST = S // 128  # 16 s-tiles of 128
NHO = HID // 512  # 8 output column tiles of 512
SCALE = 1.0 / float(np.sqrt(D))

_CACHE: dict = {}


def _build_nc():
    nc = bacc.Bacc(None, target_bir_lowering=False, debug=False)

    hst_d = nc.dram_tensor("hst", [NQR, 128, KC, 512], BF16, kind="ExternalInput")
    wq_d = nc.dram_tensor("wq", [NQ, 128, KC, D], BF16, kind="ExternalInput")
    wk_d = nc.dram_tensor("wk", [128, KC, D], BF16, kind="ExternalInput")
    wv_d = nc.dram_tensor("wv", [128, KC, D], BF16, kind="ExternalInput")
    wo_d = nc.dram_tensor("wo", [128, NQ, HID], BF16, kind="ExternalInput")
    cos_d = nc.dram_tensor("cos2", [128, S], F32, kind="ExternalInput")
    sin_d = nc.dram_tensor("sin2", [128, S], F32, kind="ExternalInput")
    out_d = nc.dram_tensor("out", [S, HID], F32, kind="ExternalOutput")

    with tile.TileContext(nc) as tc, ExitStack() as stack:
        # ---- pools that live the whole kernel ----
        const = stack.enter_context(tc.tile_pool(name="const", bufs=1))
        act = stack.enter_context(tc.tile_pool(name="act", bufs=1))
        qt_sb = [
            act.tile([128, S], BF16, tag=f"qt{h}", name=f"qt{h}") for h in range(NQ)
        ]
        kt_sb = act.tile([128, S], BF16, tag="kt")
        vt_sb = act.tile([128, S], BF16, tag="vt")
        v_sb = act.tile([128, NST, 128], BF16, tag="v")  # [s,d] chunks per k-tile
        attn_sb = [
            act.tile([128, S], BF16, tag=f"attn{h}", name=f"attn{h}")
            for h in range(NQ)
        ]
        # B-phase SBUF pools are allocated up-front (NOT from space reused
        # from the A-phase pools) — otherwise the first B eviction picks up
        # a WAR dependency on the tail of phase A and the PE/DVE/GPSIMD
        # engines form a multi-microsecond circular stall.
        probs_p = stack.enter_context(tc.tile_pool(name="probs", bufs=3))
        den_p = stack.enter_context(tc.tile_pool(name="den", bufs=2))
        bcast_p = stack.enter_context(tc.tile_pool(name="bcast", bufs=2))

        # ================= Phase A: QKV projections + RoPE =================
        with (
            tc.tile_pool(name="wqkv", bufs=1) as wqkv,
            tc.tile_pool(name="hstp", bufs=2) as hstp,
            tc.tile_pool(name="rope", bufs=2) as rope,
            tc.tile_pool(name="psA", bufs=3, space="PSUM") as psA,
            tc.tile_pool(name="psT", bufs=2, space="PSUM") as psT,
        ):
            # DMA order matters at startup: get hst[0] + wk + rope tables in
            # first so the k-projection (first job) can start ASAP.
            hst_tiles = []
            # wk (1MB) first: the k-projection's c=0 matmul needs it plus
            # only the first hst region, so PE starts ~8us earlier than
            # with wk queued behind the full 4MB hst[0] transfer.
            wk_sb = wqkv.tile([128, KC, D], BF16)
            nc.sync.dma_start(wk_sb[:], wk_d[:])
            hst_t0 = hstp.tile([128, KC, 512], BF16, tag="hst", name="hst0")
            for r in range(4):
                nc.sync.dma_start(
                    hst_t0[:, r * 8 : (r + 1) * 8, :],
                    hst_d[0, :, r * 8 : (r + 1) * 8, :],
                )
            hst_tiles.append(hst_t0)
            wv_sb = wqkv.tile([128, KC, D], BF16)
            nc.sync.dma_start(wv_sb[:], wv_d[:])
            cos_sb = const.tile([128, S], F32)
            sin_sb = const.tile([128, S], F32)
            nc.sync.dma_start(cos_sb[:], cos_d[:])
            nc.sync.dma_start(sin_sb[:], sin_d[:])
            wq_sb = [
                wqkv.tile([128, KC, D], BF16, tag=f"wq{h}", name=f"wq{h}")
                for h in range(NQ)
            ]
            for h in range(NQ):
                nc.sync.dma_start(wq_sb[h][:], wq_d[h])

            identity = const.tile([128, 128], BF16)
            make_identity(nc, identity[:])
            ones = const.tile([128, 1], BF16)
            nc.gpsimd.memset(ones[:], 1.0)
            ones_f = const.tile([128, 1], F32)
            nc.gpsimd.memset(ones_f[:], 1.0)
            # triangular mask for the diagonal 128x128 subtile: rows are k,
            # cols are q; keep q >= k.
            tri = const.tile([128, 128], BF16)
            nc.gpsimd.memset(tri[:], 1.0)
            nc.gpsimd.affine_select(
                out=tri[:],
                in_=tri[:],
                pattern=[[1, 128]],
                compare_op=mybir.AluOpType.is_ge,
                fill=0.0,
                base=0,
                channel_multiplier=-1,
            )

            def rope_evict(ps, dst_tile, qr):
                """dst[0:64]  = x0*cos - x1*sin
                dst[64:128] = x1*cos + x0*sin   (x0=ps[0:64], x1=ps[64:128])"""
                sl = slice(qr * 512, (qr + 1) * 512)
                raw = rope.tile([128, 512], F32, tag="raw")
                nc.vector.tensor_copy(raw[:], ps[:])
                swp = rope.tile([128, 512], F32, tag="swp")
                nc.sync.dma_start(swp[0:64, :], raw[64:128, :])
                nc.sync.dma_start(swp[64:128, :], raw[0:64, :])
                # in-place: raw *= cos, swp *= sin
                nc.vector.tensor_mul(raw[:], raw[:], cos_sb[:, sl])
                nc.vector.tensor_mul(swp[:], swp[:], sin_sb[:, sl])
                nc.vector.tensor_sub(dst_tile[0:64, sl], raw[0:64, :], swp[0:64, :])
                nc.vector.tensor_add(
                    dst_tile[64:128, sl], raw[64:128, :], swp[64:128, :]
                )

            for qr in range(NQR):
                if qr + 1 < NQR:
                    nxt = hstp.tile([128, KC, 512], BF16, tag="hst", name=f"hst{qr+1}")
                    nc.sync.dma_start(nxt[:], hst_d[qr + 1])
                    hst_tiles.append(nxt)
                hst_t = hst_tiles[qr]
                # k and v first (their weights arrive first)
                jobs = [("k", 0), ("v", 0)] + [("q", h) for h in range(NQ)]
                for kind, h in jobs:
                    ps = psA.tile([128, 512], F32)
                    for c in range(KC):
                        if kind == "q":
                            lhsT = wq_sb[h][:, c, :]
                        elif kind == "k":
                            lhsT = wk_sb[:, c, :]
                        else:
                            lhsT = wv_sb[:, c, :]
                        nc.tensor.matmul(
                            ps[:],
                            lhsT,
                            hst_t[:, c, :],
                            start=(c == 0),
                            stop=(c == KC - 1),
                        )
                    if kind == "q":
                        rope_evict(ps, qt_sb[h], qr)
                    elif kind == "k":
                        rope_evict(ps, kt_sb, qr)
                    else:
                        sl = slice(qr * 512, (qr + 1) * 512)
                        nc.vector.tensor_copy(vt_sb[:, sl], ps[:])
                # transpose this qr's v slice into [s, d] chunks
                for kt in range(qr * 4, qr * 4 + 4):
                    pst = psT.tile([128, 128], BF16)
                    nc.tensor.transpose(
                        pst[:], vt_sb[:, kt * 128 : (kt + 1) * 128], identity[:]
                    )
                    nc.vector.tensor_copy(v_sb[:, kt, :], pst[:])

        # ========== Phase B: causal attention (+ interleaved o_proj) ==========
        wo_pool = stack.enter_context(tc.tile_pool(name="wo", bufs=1))
        wo_sb = wo_pool.tile([128, NQ, HID], BF16)
        nc.sync.dma_start(wo_sb[:], wo_d[:])
        ostage = stack.enter_context(tc.tile_pool(name="ostage", bufs=6))

        # k-tiles are processed in PAIRS: one [128, 1024] PSUM scores tile
        # holds two k-tiles' scoresT so one ACT exp instruction covers both
        # (the 352-cycle per-ACTIVATE overhead would otherwise make ACT the
        # phase-B pacer). Diagonal pairs use two column-narrowed exps.
        #
        # Phase B is exp-throughput-paced on ACT, leaving PE bubbles; o_proj
        # (phase C) matmuls for already-completed q-ranges are interleaved
        # into those bubbles as PE filler. PSUM: scores pairs 2x2 banks +
        # o_proj accumulators 2 + attention output 2 = 8 banks.
        with (
            tc.tile_pool(name="psS", bufs=2, space="PSUM") as psS,
            tc.tile_pool(name="psO", bufs=2, space="PSUM") as psO,
        ):
            # ---- o_proj work-unit generator ----
            def c_units():
                for qrC in range(NQR):
                    for st in range(qrC * 4, qrC * 4 + 4):
                        for ho in range(NHO):
                            yield ("alloc", qrC, st, ho)
                            for h in range(NQ):
                                yield ("mm", qrC, st, ho, h)
                            yield ("evict", qrC, st, ho)

            c_state = {"gen": c_units(), "pending": None, "tile": None,
                       "done": set(), "open": None}

            def emit_c(n_mms, qr_done, evict_engine="v"):
                emitted = 0
                while emitted < n_mms:
                    unit = c_state["pending"] or next(c_state["gen"], None)
                    c_state["pending"] = None
                    if unit is None:
                        return False
                    if unit[1] > qr_done:
                        c_state["pending"] = unit
                        return False
                    if unit[0] == "alloc":
                        _, _, st, ho = unit
                        c_state["open"] = (st, ho)
                        c_state["tile"] = psS.tile(
                            [128, 512], F32, tag="c", bufs=2, name=f"c{st}_{ho}"
                        )
                    elif unit[0] == "mm":
                        _, _, st, ho, h = unit
                        nc.tensor.matmul(
                            c_state["tile"][:],
                            attn_sb[h][:, st * 128 : (st + 1) * 128],
                            wo_sb[:, h, ho * 512 : (ho + 1) * 512],
                            start=(h == 0),
                            stop=(h == NQ - 1),
                            skip_group_check=True,
                        )
                        emitted += 1
                    else:
                        _, _, st, ho = unit
                        c_state["done"].add((st, ho))
                        stg = ostage.tile([128, 512], F32, tag="stg")
                        if evict_engine == "v":
                            nc.vector.tensor_copy(stg[:], c_state["tile"][:])
                        else:
                            nc.scalar.copy(stg[:], c_state["tile"][:])
                        nc.sync.dma_start(
                            out_d[
                                st * 128 : (st + 1) * 128,
                                ho * 512 : (ho + 1) * 512,
                            ],
                            stg[:],
                        )
                return True

            for qr in range(NQR):
                n_kt = 4 * (qr + 1)
                n_pair = n_kt // 2
                qsl = slice(qr * 512, (qr + 1) * 512)

                def c0_of(kt, qr=qr):
                    p_idx = kt - 4 * qr
                    return 128 * p_idx if p_idx > 0 else 0

                for h in range(NQ):
                    ps_o = psO.tile([128, 512], F32, tag="o", name=f"o{qr}_{h}")
                    den_acc = den_p.tile(
                        [128, 512], F32, tag="da", name=f"da{qr}_{h}"
                    )
                    pair_tiles = {}

                    def mm_scores_pair(j, qr=qr, h=h, pair_tiles=pair_tiles):
                        ps_s = psS.tile(
                            [128, 1024], F32, tag="s", name=f"s{qr}_{h}_{j}"
                        )
                        for idx in range(2):
                            kt = 2 * j + idx
                            c0 = c0_of(kt)
                            nc.tensor.matmul(
                                ps_s[:, idx * 512 + c0 : (idx + 1) * 512],
                                kt_sb[:, kt * 128 : (kt + 1) * 128],
                                qt_sb[h][:, qr * 512 + c0 : (qr + 1) * 512],
                                start=True,
                                stop=True,
                            )
                        pair_tiles[j] = ps_s

                    mm_scores_pair(0)
                    for j in range(n_pair):
                        if j + 1 < n_pair:
                            mm_scores_pair(j + 1)
                        emit_c(3, qr - 1)
                        ps_s = pair_tiles.pop(j)
                        pt = probs_p.tile(
                            [128, 1024], BF16, tag="pt", name=f"pt{qr}_{h}_{j}"
                        )
                        if 2 * j + 1 < 4 * qr:
                            # fully off-diagonal pair: one batched exp
                            nc.scalar.activation(
                                pt[:],
                                ps_s[:],
                                mybir.ActivationFunctionType.Exp,
                                scale=SCALE,
                            )
                        else:
                            for idx in range(2):
                                kt = 2 * j + idx
                                c0 = c0_of(kt)
                                nc.scalar.activation(
                                    pt[:, idx * 512 + c0 : (idx + 1) * 512],
                                    ps_s[:, idx * 512 + c0 : (idx + 1) * 512],
                                    mybir.ActivationFunctionType.Exp,
                                    scale=SCALE,
                                )
                                # triangular mask on the diagonal subtile
                                nc.vector.tensor_mul(
                                    pt[:, idx * 512 + c0 : idx * 512 + c0 + 128],
                                    pt[:, idx * 512 + c0 : idx * 512 + c0 + 128],
                                    tri[:],
                                )
                        for idx in range(2):
                            kt = 2 * j + idx
                            c0 = c0_of(kt)
                            nc.tensor.matmul(
                                ps_o[:, c0:512],
                                v_sb[:, kt, :],
                                pt[:, idx * 512 + c0 : (idx + 1) * 512],
                                start=(kt == 0),
                                stop=(kt == n_kt - 1),
                                skip_group_check=True,
                            )
                        # denominator: fold the pair halves (bf16) and
                        # accumulate on DVE in f32 — no per-pair PE matmul
                        pts = probs_p.tile(
                            [128, 512], BF16, tag="pts", name=f"pts{qr}_{h}_{j}"
                        )
                        c0a, c0b = c0_of(2 * j), c0_of(2 * j + 1)
                        if c0b > c0a:
                            nc.vector.tensor_copy(
                                pts[:, c0a:c0b], pt[:, c0a:c0b]
                            )
                        nc.vector.tensor_add(
                            pts[:, c0b:512],
                            pt[:, c0b:512],
                            pt[:, 512 + c0b : 1024],
                        )
                        if j == 0:
                            nc.vector.tensor_copy(den_acc[:], pts[:])
                        else:
                            nc.vector.tensor_add(
                                den_acc[:, c0a:512],
                                den_acc[:, c0a:512],
                                pts[:, c0a:512],
                            )
                    emit_c(2, qr - 1)
                    # partition-reduce den_acc with one fp32 matmul into the
                    # (dead) last scores psum tile, then recip + broadcast
                    nc.tensor.matmul(
                        ps_s[0:1, 0:512],
                        ones_f[:],
                        den_acc[:],
                        start=True,
                        stop=True,
                        skip_group_check=True,
                    )
                    recip = den_p.tile(
                        [1, 512], F32, tag="recip", name=f"rc{qr}_{h}"
                    )
                    nc.vector.reciprocal_approx_fast(
                        out=recip[:], in_=ps_s[0:1, 0:512]
                    )
                    bc = bcast_p.tile([128, 512], F32, tag="bc")
                    nc.gpsimd.partition_broadcast(bc[:], recip[:])
                    nc.vector.tensor_mul(attn_sb[h][:, qsl], ps_o[:], bc[:])

            # ---- drain the remaining o_proj work ----
            alt = 0
            while emit_c(4, NQR - 1, evict_engine=("v" if alt % 2 == 0 else "s")):
                alt += 1

    nc.compile()
    return nc


def _get_nc():
    if "nc" not in _CACHE:
        _CACHE["nc"] = _build_nc()
    return _CACHE["nc"]


def _bf16(x):
    return np.ascontiguousarray(x.astype(ml_dtypes.bfloat16))


def _prep_in_maps(hidden_states, sin_table, cos_table, Wq, Wk, Wv, Wo):
    hs0 = np.asarray(hidden_states, np.float32).reshape(S, HID)
    # hst[qr, p, c, s] = hs0[qr*512 + s, c*128 + p]
    hst = _bf16(hs0.reshape(NQR, 512, KC, 128).transpose(0, 3, 2, 1))
    cosT = np.asarray(cos_table, np.float32).T  # [64, S]
    sinT = np.asarray(sin_table, np.float32).T
    cos2 = np.ascontiguousarray(np.concatenate([cosT, cosT], 0))  # [128, S]
    sin2 = np.ascontiguousarray(np.concatenate([sinT, sinT], 0))
    Wq = np.asarray(Wq, np.float32)
    Wk = np.asarray(Wk, np.float32)
    Wv = np.asarray(Wv, np.float32)
    Wo = np.asarray(Wo, np.float32)

    in_maps = []
    for c in range(N_CORES):
        wq_c = Wq[:, c * 512 : (c + 1) * 512]  # 4 q heads
        wk_c = Wk[:, c * 128 : (c + 1) * 128]  # 1 kv head
        wv_c = Wv[:, c * 128 : (c + 1) * 128]
        wo_c = Wo[c * 512 : (c + 1) * 512, :]  # matching rows
        # wq per-head-major: [h, p, c, d] with element Wq_c[c*128+p, h*128+d]
        wq_l = wq_c.reshape(KC, 128, NQ, D).transpose(2, 1, 0, 3)
        in_maps.append(
            {
                "hst": hst,
                "wq": _bf16(wq_l),
                "wk": _bf16(wk_c.reshape(KC, 128, D).swapaxes(0, 1)),
                "wv": _bf16(wv_c.reshape(KC, 128, D).swapaxes(0, 1)),
                "wo": _bf16(wo_c.reshape(NQ, 128, HID).swapaxes(0, 1)),
                "cos2": cos2,
                "sin2": sin2,
            }
        )
    return in_maps


def run(trace=False, **inputs):
    nc = _get_nc()
    in_maps = _prep_in_maps(**inputs)
    res = run_bass_kernel_spmd(
        nc, in_maps, core_ids=list(range(N_CORES)), trace=trace
    )
    partials = np.stack([res.results[c]["out"] for c in range(N_CORES)])
    out = partials.sum(axis=0, dtype=np.float32).reshape(1, S, HID)
    return out, res


def kernel(**inputs):
    out, _ = run(trace=False, **inputs)
    return out
